# revision 1
# baseline (speedup 1.0000x reference)
"""Trainium2 Bass kernel for nn_ClusterEncoder (PointTransformerConv-style
GNN message passing), 8-core SPMD.

The metric regime is axon-tunnel transfer-bound (~21 MB/s host<->device),
so the kernel minimizes per-call host bytes:
  * x is NOT replicated: each core receives a transposed bf16 shard
    xT_sh = x[c*6250:(c+1)*6250].T  ([128, 6250], 1.6 MB/core) and the
    full xT is assembled on device with an AllGather collective.
  * Edge index tensors ship as uint16 (node ids < 50000) / int8
    (chunk-local dst slot), expanded to i32/f32 on device.
  * posdT ships as int8 (|posd|<1; the 1/127 dequant scale is folded
    into Wp1 host-side), weights ship as bf16 bit patterns.
  * y returns as u8 (scale 6/255; abs quant err ~0.012 vs 0.049 budget),
    decoded on host. bf16/f32 fallbacks behind Cfg.y_mode.

Compute strategy (edges sharded by destination node):
  * Host: sort edges by dst, split nodes into 8 equal contiguous ranges.
    Within a core, greedy-pack destination nodes into "chunks" of <=128
    nodes and <=CHUNK_E edges; pad each chunk's edge list to CHUNK_E.
  * Device, phase 0: AllGather xT shards -> xT_all [8*128, 6250]
    (block c = x[c*6250:(c+1)*6250].T).
  * Device, phase 1 (replicated): U = x @ (W_dst@Wa1),
    VH = x @ [W_src@Wa1 | W_lin] into DRAM; row-gatherable layouts.
    Uses xT_all blocks directly as matmul lhsT (no on-device transpose).
  * Device, phase 2 (per chunk of 16 x 128-edge tiles):
      - gather VH rows by src (768B/row) and U rows by dst,
      - gd = U[dst] - V[src]  (attn-layer-1 folded through node features),
      - pos MLP: t_p1 = relu(Wp1^T posd^T + bp1), delta = relu(Wp2^T t_p1 + bp2),
      - z1 = Wa1^T delta;  t_a = relu(z1 + gd^T + ba1),
      - logits = relu(Wa2^T t_a + ba2);  e = exp(logits - SHIFT)
        (softmax max-subtraction replaced by a constant shift -- exactly
        equivalent math since the shift cancels in e/sum(e); logits are
        relu-bounded so no overflow),
      - one-hot indicator per tile from local dst index (iota + is_equal),
      - segment-sum via matmul: acc[n, 0:128] += ind^T @ (e*(H[src]+delta))^T,
        acc[n, 128:256] += ind^T @ e^T   (numerator and normalizer together),
      - out = relu(NUM / (s + eps)); indirect-scatter rows to y (u8).
  * Softmax segments are core-local by construction (edges sharded by dst).
"""
import sys
from dataclasses import dataclass
from math import ceil

if "/opt/trn_rl_repo" not in sys.path:
    sys.path.insert(0, "/opt/trn_rl_repo")

import ml_dtypes
import numpy as np

import concourse.bass as bass
import concourse.mybir as mybir
import concourse.tile as tile
from concourse import bacc
from concourse.bass import IndirectOffsetOnAxis
from concourse.bass_utils import run_bass_kernel_spmd
from concourse.masks import make_identity

f32 = mybir.dt.float32
f32r = mybir.dt.float32r
bf16 = mybir.dt.bfloat16
i32 = mybir.dt.int32
u16 = mybir.dt.uint16
i8 = mybir.dt.int8
AF = mybir.ActivationFunctionType
ALU = mybir.AluOpType
BF16 = ml_dtypes.bfloat16


@dataclass
class Cfg:
    N: int = 50000
    C: int = 128
    PH: int = 64
    AH: int = 64
    DIM: int = 2
    M: int = 8            # cores
    T: int = 16           # 128-edge tiles per chunk
    TB: int = 4           # tiles per matmul block (block = 512 edges)
    SHIFT: float = 8.0
    EPS: float = 1e-12
    mm_dt: object = f32r  # matmul compute dtype (f32r: 1 cyc/row at free>=256)
    y_mode: str = "u8"    # "f32" | "bf16" | "u8" output encoding
    Y_SCALE: float = 6.0 / 255.0  # u8 quant step (y in [0, ~2.44]; 2.4x margin)
    pd_bf16_mm: bool = True   # feed posdT bf16 tile straight into matmul
    p1_bf16_mm: bool = False  # phase-1 matmul in bf16 (crashes exec unit; keep off)
    w_bf16: bool = True   # ship matmul weights as bf16

    @property
    def NLOC(self):
        return self.N // self.M

    @property
    def NSH(self):
        return self.N // self.M  # x rows per core shard

    @property
    def CHUNK_E(self):
        return self.T * 128

    @property
    def OUT_ROWS(self):
        return self.NLOC + 1  # +1 trash row for padded scatter lanes


CFG = Cfg()


def _to_bf16_bits(a):
    return np.ascontiguousarray(a.astype(BF16).view(np.uint16))


# ---------------------------------------------------------------- host pack
def _pack(x, pos, edge_index, cfg):
    """Sort/shard/chunk edges; returns per-core input dicts (minus weights)."""
    src = np.asarray(edge_index[0], np.int64)
    dst = np.asarray(edge_index[1], np.int64)
    order = np.argsort(dst, kind="stable")
    s_s = src[order]
    d_s = dst[order]
    posd = (pos[d_s] - pos[s_s]).astype(np.float32)  # [E, 2]

    NLOC = cfg.NLOC
    bounds = np.searchsorted(d_s, np.arange(cfg.M + 1) * NLOC)

    cores = []
    for c in range(cfg.M):
        lo, hi = bounds[c], bounds[c + 1]
        dloc = d_s[lo:hi] - c * NLOC
        deg = np.bincount(dloc, minlength=NLOC)
        nodes = np.nonzero(deg)[0]
        chunks = []  # (node_list, e0, e1) ; e relative to lo
        cur, cur_e, estart = [], 0, 0
        for n in nodes:
            dn = int(deg[n])
            assert dn <= cfg.CHUNK_E, f"degree {dn} exceeds chunk capacity"
            if len(cur) == 128 or cur_e + dn > cfg.CHUNK_E:
                chunks.append((cur, estart, estart + cur_e))
                estart += cur_e
                cur, cur_e = [], 0
            cur.append(int(n))
            cur_e += dn
        if cur:
            chunks.append((cur, estart, estart + cur_e))
        cores.append((lo, chunks, dloc))

    NCHUNK = max(len(ch) for _, ch, _ in cores) if cores else 1
    NCHUNK = max(NCHUNK, 1)

    in_maps = []
    for c in range(cfg.M):
        lo, chunks, dloc = cores[c]
        srcid = np.zeros((NCHUNK, 128, cfg.T), np.uint16)
        dstid = np.zeros((NCHUNK, 128, cfg.T), np.uint16)
        dstloc = np.full((NCHUNK, 128, cfg.T), -1, np.int8)
        posdT = np.zeros((NCHUNK, cfg.DIM, cfg.CHUNK_E), np.float32)
        outrow = np.full((NCHUNK, 128), cfg.NLOC, np.uint16)  # trash row
        for k, (nl, e0, e1) in enumerate(chunks):
            cnt = e1 - e0
            g0, g1 = lo + e0, lo + e1
            nla = np.asarray(nl, np.int64)
            loc = np.searchsorted(nla, dloc[e0:e1]).astype(np.int8)
            j = np.arange(cnt)
            t_idx = j >> 7
            lane = j & 127
            srcid[k, lane, t_idx] = s_s[g0:g1].astype(np.uint16)
            dstid[k, lane, t_idx] = d_s[g0:g1].astype(np.uint16)
            dstloc[k, lane, t_idx] = loc
            posdT[k, :, :cnt] = posd[g0:g1].T
            outrow[k, : len(nl)] = nla.astype(np.uint16)
        pq = np.clip(np.round(posdT * 127.0), -127, 127).astype(np.int8)
        in_maps.append(
            dict(srcid=srcid, dstid=dstid, dstloc=dstloc,
                 posdT=pq, outrow=outrow)
        )
    return in_maps, NCHUNK


# ---------------------------------------------------------------- program
def _build(cfg, nchunk):
    nc = bacc.Bacc(None, target_bir_lowering=False, num_devices=cfg.M)
    N, C, PH, AH, DIM = cfg.N, cfg.C, cfg.PH, cfg.AH, cfg.DIM
    NSH = cfg.NSH
    mdt = cfg.mm_dt

    w_dt = bf16 if cfg.w_bf16 else f32
    xts_d = nc.declare_dram_parameter("xTsh", [C, NSH], bf16, isOutput=False)
    wnode_d = nc.declare_dram_parameter("Wnode", [C, 2 * AH + C], w_dt, isOutput=False)
    wp1_d = nc.declare_dram_parameter("Wp1", [DIM, PH], f32, isOutput=False)
    wp2_d = nc.declare_dram_parameter("Wp2", [PH, C], w_dt, isOutput=False)
    wa1_d = nc.declare_dram_parameter("Wa1p", [C, AH], w_dt, isOutput=False)
    wa2_d = nc.declare_dram_parameter("Wa2", [AH, C], w_dt, isOutput=False)
    bias_d = nc.declare_dram_parameter("bias", [128, 6], f32, isOutput=False)
    src_d = nc.declare_dram_parameter("srcid", [nchunk, 128, cfg.T], u16, isOutput=False)
    dst_d = nc.declare_dram_parameter("dstid", [nchunk, 128, cfg.T], u16, isOutput=False)
    dl_d = nc.declare_dram_parameter("dstloc", [nchunk, 128, cfg.T], i8, isOutput=False)
    pd_d = nc.declare_dram_parameter("posdT", [nchunk, DIM, cfg.CHUNK_E], i8, isOutput=False)
    or_d = nc.declare_dram_parameter("outrow", [nchunk, 128], u16, isOutput=False)
    y_dt = {"f32": f32, "bf16": bf16, "u8": mybir.dt.uint8}[cfg.y_mode]
    y_d = nc.declare_dram_parameter("y", [cfg.OUT_ROWS, C], y_dt, isOutput=True)

    U_d = nc.dram_tensor("U", [N, AH], f32)          # x @ (W_dst@Wa1)
    VH_d = nc.dram_tensor("VH", [N, AH + C], f32)    # x @ [W_src@Wa1 | W_lin]
    xg_in = nc.dram_tensor("xg_in", [C, NSH], bf16)  # collective input bounce
    xT_all = nc.dram_tensor("xT_all", [cfg.M * C, NSH], bf16)

    NB = cfg.T // cfg.TB  # blocks per chunk
    BLK = cfg.TB * 128

    with tile.TileContext(nc) as tc:
        with tc.tile_pool(name="const", bufs=1) as cp:
            wnode_s = cp.tile([C, 2 * AH + C], w_dt)
            nc.sync.dma_start(out=wnode_s[:], in_=wnode_d[:, :])
            wp1_s = cp.tile([DIM, PH], f32)
            nc.sync.dma_start(out=wp1_s[:], in_=wp1_d[:, :])
            wp2_s = cp.tile([PH, C], w_dt)
            nc.sync.dma_start(out=wp2_s[:], in_=wp2_d[:, :])
            wa2_s = cp.tile([AH, C], w_dt)
            nc.sync.dma_start(out=wa2_s[:], in_=wa2_d[:, :])
            bias_s = cp.tile([128, 6], f32)
            nc.sync.dma_start(out=bias_s[:], in_=bias_d[:, :])
            ident_s = cp.tile([128, 128], f32)
            make_identity(nc, ident_s[:])
            iota_i = cp.tile([128, 128], i32)
            nc.gpsimd.iota(iota_i[:], pattern=[[1, 128]], base=0, channel_multiplier=0)
            iota_s = cp.tile([128, 128], f32)
            nc.vector.tensor_copy(iota_s[:], iota_i[:])
            wa1_s = cp.tile([C, AH], w_dt)
            nc.sync.dma_start(out=wa1_s[:], in_=wa1_d[:, :])

            # phase-1 weights in bf16 (match bf16 x operand)
            p1_dt = bf16 if cfg.p1_bf16_mm else mdt
            wnode_b = cp.tile([C, 2 * AH + C], p1_dt)
            nc.vector.tensor_copy(wnode_b[:], wnode_s[:])
            # pos-MLP layer-1 weights: match the pd operand dtype
            pd_mm_dt = bf16 if cfg.pd_bf16_mm else mdt
            wp1_b = cp.tile([DIM, PH], pd_mm_dt)
            nc.vector.tensor_copy(wp1_b[:], wp1_s[:])

            # fp32r matmul operands must be produced rounded-to-f32r: make
            # rounded copies of the stationary weights once.
            if mdt is f32r:
                wp2_m = cp.tile([PH, C], f32r)
                nc.vector.tensor_copy(wp2_m[:], wp2_s[:])
                wa1_m = cp.tile([C, AH], f32r)
                nc.vector.tensor_copy(wa1_m[:], wa1_s[:])
                wa2_m = cp.tile([AH, C], f32r)
                nc.vector.tensor_copy(wa2_m[:], wa2_s[:])
            else:
                wp2_m, wa1_m, wa2_m = wp2_s, wa1_s, wa2_s

            # ---------------- phase 0: AllGather x shards ----------------
            nc.gpsimd.dma_start(out=xg_in[:, :], in_=xts_d[:, :])
            nc.gpsimd.collective_compute(
                "AllGather",
                mybir.AluOpType.bypass,
                replica_groups=[list(range(cfg.M))],
                ins=[xg_in[:, :]],
                outs=[xT_all[:, :]],
            )

            # ---------------- phase 1: node features U / VH ----------------
            # xT_all block c8 = x[c8*NSH:(c8+1)*NSH].T  -> use as matmul lhsT
            with tc.tile_pool(name="p1", bufs=3) as p1, \
                 tc.tile_pool(name="p1ps", bufs=2, space="PSUM") as p1ps:
                ncol = ceil(NSH / 128)
                for c8 in range(cfg.M):
                    for t in range(ncol):
                        j0 = t * 128
                        rows = min(128, NSH - j0)
                        r0 = c8 * NSH + j0
                        xt = p1.tile([C, 128], bf16, tag="xt")
                        nc.sync.dma_start(
                            out=xt[:, :rows],
                            in_=xT_all[c8 * C:(c8 + 1) * C, j0:j0 + rows])
                        if cfg.p1_bf16_mm:
                            xt_m = xt
                        else:
                            xt_m = p1.tile([C, 128], mdt, tag="xtm")
                            nc.vector.tensor_copy(xt_m[:, :rows], xt[:, :rows])
                        uvh_p = p1ps.tile([128, 2 * AH + C], f32, tag="uvh")
                        nc.tensor.matmul(uvh_p[:rows, :], lhsT=xt_m[:, :rows],
                                         rhs=wnode_b[:], start=True, stop=True)
                        uvh_s = p1.tile([128, 2 * AH + C], f32, tag="uvhs")
                        nc.scalar.activation(uvh_s[:rows, :], uvh_p[:rows, :], AF.Copy)
                        nc.sync.dma_start(out=U_d[r0:r0 + rows, :], in_=uvh_s[:rows, 0:AH])
                        nc.sync.dma_start(out=VH_d[r0:r0 + rows, :], in_=uvh_s[:rows, AH:])

            # ---------------- phase 2: edges ----------------
            with tc.tile_pool(name="eb", bufs=3) as eb, \
                 tc.tile_pool(name="ebg", bufs=3) as ebg, \
                 tc.tile_pool(name="ps_acc", bufs=2, space="PSUM") as ps_acc, \
                 tc.tile_pool(name="ps_b", bufs=1, space="PSUM") as ps_b, \
                 tc.tile_pool(name="ps_c", bufs=1, space="PSUM") as ps_c, \
                 tc.tile_pool(name="ps_m", bufs=1, space="PSUM") as ps_m, \
                 tc.tile_pool(name="ps_n", bufs=1, space="PSUM") as ps_n, \
                 tc.tile_pool(name="ps_t", bufs=2, space="PSUM") as ps_t:
                for k in range(nchunk):
                    src16 = eb.tile([128, cfg.T], u16, tag="src16")
                    nc.sync.dma_start(out=src16[:], in_=src_d[k, :, :])
                    src_s = eb.tile([128, cfg.T], i32, tag="src")
                    nc.vector.tensor_copy(src_s[:], src16[:])
                    dst16 = eb.tile([128, cfg.T], u16, tag="dst16")
                    nc.sync.dma_start(out=dst16[:], in_=dst_d[k, :, :])
                    dst_s = eb.tile([128, cfg.T], i32, tag="dst")
                    nc.vector.tensor_copy(dst_s[:], dst16[:])
                    dl8 = eb.tile([128, cfg.T], i8, tag="dl8")
                    nc.sync.dma_start(out=dl8[:], in_=dl_d[k, :, :])
                    dl_s = eb.tile([128, cfg.T], f32, tag="dl")
                    nc.vector.tensor_copy(dl_s[:], dl8[:])
                    pd_s = eb.tile([DIM, cfg.CHUNK_E], i8, tag="pd")
                    nc.sync.dma_start(out=pd_s[:], in_=pd_d[k, :, :])
                    pd_mm = bf16 if cfg.pd_bf16_mm else mdt
                    pd_m = eb.tile([DIM, cfg.CHUNK_E], pd_mm, tag="pdm")
                    nc.vector.tensor_copy(pd_m[:], pd_s[:])
                    or16 = eb.tile([128, 1], u16, tag="or16")
                    nc.sync.dma_start(out=or16[:], in_=or_d[k, :, None])
                    or_s = eb.tile([128, 1], i32, tag="or")
                    nc.vector.tensor_copy(or_s[:], or16[:])

                    acc_p = ps_acc.tile([128, 2 * C], f32, tag="acc")

                    for b in range(NB):
                        esl = slice(b * BLK, (b + 1) * BLK)
                        # gathers for this block, one [128,1]-offset DMA per tile
                        vhgs, ugs = [], []
                        for tt in range(cfg.TB):
                            ti = b * cfg.TB + tt
                            vhg_t = ebg.tile([128, AH + C], f32, tag=f"vhg{tt}")
                            nc.gpsimd.indirect_dma_start(
                                out=vhg_t[:], out_offset=None, in_=VH_d[:],
                                in_offset=IndirectOffsetOnAxis(
                                    ap=src_s[:, ti:ti + 1], axis=0))
                            vhgs.append(vhg_t)
                            ug_t = ebg.tile([128, AH], f32, tag=f"ug{tt}")
                            nc.gpsimd.indirect_dma_start(
                                out=ug_t[:], out_offset=None, in_=U_d[:],
                                in_offset=IndirectOffsetOnAxis(
                                    ap=dst_s[:, ti:ti + 1], axis=0))
                            ugs.append(ug_t)

                        # pos MLP
                        tp1_p = ps_m.tile([PH, BLK], f32, tag="tp1")
                        nc.tensor.matmul(tp1_p[:], lhsT=wp1_b[:],
                                         rhs=pd_m[:, esl], start=True, stop=True)
                        tp1_s = eb.tile([PH, BLK], mdt, tag="tp1s")
                        nc.scalar.activation(tp1_s[:], tp1_p[:], AF.Relu, bias=bias_s[0:PH, 0:1])
                        del_p = ps_b.tile([C, BLK], f32, tag="delp")
                        nc.tensor.matmul(del_p[:], lhsT=wp2_m[:],
                                         rhs=tp1_s[:], start=True, stop=True)
                        del_s = eb.tile([C, BLK], f32, tag="dels")
                        nc.scalar.activation(del_s[:], del_p[:], AF.Relu, bias=bias_s[:, 1:2])
                        if mdt is f32r:
                            del_m = eb.tile([C, BLK], f32r, tag="delm")
                            nc.scalar.activation(del_m[:], del_p[:], AF.Relu, bias=bias_s[:, 1:2])
                        else:
                            del_m = del_s

                        # attn layer 1: z1 = Wa1^T delta ; t_a = relu(z1 + gd^T + ba1)
                        z1_p = ps_n.tile([AH, BLK], f32, tag="z1")
                        nc.tensor.matmul(z1_p[:], lhsT=wa1_m[:],
                                         rhs=del_m[:], start=True, stop=True)
                        tsum_s = eb.tile([AH, BLK], f32, tag="tsum")
                        for tt in range(cfg.TB):
                            gd_s = eb.tile([128, AH], f32, tag="gd")
                            nc.vector.tensor_tensor(gd_s[:], ugs[tt][:], vhgs[tt][:, 0:AH],
                                                    op=ALU.subtract)
                            gdT_p = ps_t.tile([128, 128], f32, tag="tr")
                            nc.tensor.transpose(gdT_p[:AH, :], gd_s[:], ident_s[:])
                            gdT_s = eb.tile([AH, 128], f32, tag="gdT")
                            nc.scalar.activation(gdT_s[:], gdT_p[:AH, :], AF.Copy)
                            csl = slice(tt * 128, (tt + 1) * 128)
                            nc.vector.tensor_tensor(tsum_s[:, csl], z1_p[:, csl],
                                                    gdT_s[:], op=ALU.add)
                        ta_s = eb.tile([AH, BLK], mdt, tag="ta")
                        nc.scalar.activation(ta_s[:], tsum_s[:], AF.Relu, bias=bias_s[0:AH, 2:3])

                        # attn layer 2 + exp
                        al_p = ps_c.tile([C, BLK], f32, tag="al")
                        nc.tensor.matmul(al_p[:], lhsT=wa2_m[:],
                                         rhs=ta_s[:], start=True, stop=True)
                        ar_s = eb.tile([C, BLK], f32, tag="ar")
                        nc.scalar.activation(ar_s[:], al_p[:], AF.Relu, bias=bias_s[:, 3:4])
                        e_s = eb.tile([C, BLK], f32, tag="e")
                        nc.scalar.activation(e_s[:], ar_s[:], AF.Exp, bias=bias_s[:, 4:5])
                        ew2_s = eb.tile([C, BLK], f32, tag="ew2")
                        nc.vector.tensor_tensor(ew2_s[:], e_s[:], del_s[:], op=ALU.mult)
                        del del_s  # f32 copy only feeds ew2

                        # per-tile: transpose, assemble [ew | e]^T, indicator, seg-matmul
                        for tt in range(cfg.TB):
                            ti = b * cfg.TB + tt
                            csl = slice(tt * 128, (tt + 1) * 128)
                            eT_p = ps_t.tile([128, 128], f32, tag="tr")
                            nc.tensor.transpose(eT_p[:], e_s[:, csl], ident_s[:])
                            ew2T_p = ps_t.tile([128, 128], f32, tag="tr")
                            nc.tensor.transpose(ew2T_p[:], ew2_s[:, csl], ident_s[:])
                            ewe_s = eb.tile([128, 2 * C], mdt, tag="ewe")
                            nc.vector.tensor_copy(ewe_s[:, C:], eT_p[:])
                            tmp_s = eb.tile([128, C], f32, tag="tmp")
                            nc.vector.tensor_tensor(tmp_s[:], eT_p[:], vhgs[tt][:, AH:],
                                                    op=ALU.mult)
                            nc.vector.tensor_tensor(ewe_s[:, 0:C], tmp_s[:], ew2T_p[:],
                                                    op=ALU.add)
                            ind_s = eb.tile([128, 128], mdt, tag="ind")
                            nc.vector.tensor_scalar(ind_s[:], iota_s[:], dl_s[:, ti:ti + 1],
                                                    None, op0=ALU.is_equal)
                            nc.tensor.matmul(acc_p[:], lhsT=ind_s[:],
                                             rhs=ewe_s[:],
                                             start=(ti == 0), stop=(ti == cfg.T - 1))

                    # finalize chunk
                    sp_s = eb.tile([128, C], f32, tag="sp")
                    nc.vector.tensor_scalar_add(sp_s[:], acc_p[:, C:], cfg.EPS)
                    rp_s = eb.tile([128, C], f32, tag="rp")
                    nc.vector.reciprocal(rp_s[:], sp_s[:])
                    o_s = eb.tile([128, C], f32, tag="o")
                    nc.vector.tensor_tensor(o_s[:], acc_p[:, 0:C], rp_s[:], op=ALU.mult)
                    o2_s = eb.tile([128, C], y_dt, tag="o2")
                    if cfg.y_mode == "u8":
                        # f32->u8 conversion rounds to nearest
                        nc.scalar.activation(o2_s[:], o_s[:], AF.Relu,
                                             scale=1.0 / cfg.Y_SCALE)
                    else:
                        nc.scalar.activation(o2_s[:], o_s[:], AF.Relu)
                    nc.gpsimd.indirect_dma_start(
                        out=y_d[:], out_offset=IndirectOffsetOnAxis(ap=or_s[:, :1], axis=0),
                        in_=o2_s[:], in_offset=None)
    nc.finalize()
    return nc


def _build_inputs(inputs, cfg):
    x = np.ascontiguousarray(np.asarray(inputs["x"], np.float32))
    pos = np.ascontiguousarray(np.asarray(inputs["pos"], np.float32))
    W_lin = np.asarray(inputs["W_lin"], np.float32)
    W_src = np.asarray(inputs["W_src"], np.float32)
    W_dst = np.asarray(inputs["W_dst"], np.float32)
    Wp1 = np.asarray(inputs["Wp1"], np.float32)
    bp1 = np.asarray(inputs["bp1"], np.float32)
    Wp2 = np.asarray(inputs["Wp2"], np.float32)
    bp2 = np.asarray(inputs["bp2"], np.float32)
    Wa1 = np.asarray(inputs["Wa1"], np.float32)
    ba1 = np.asarray(inputs["ba1"], np.float32)
    Wa2 = np.asarray(inputs["Wa2"], np.float32)
    ba2 = np.asarray(inputs["ba2"], np.float32)

    Wda = (W_dst @ Wa1).astype(np.float32)   # [C, AH]
    Wsa = (W_src @ Wa1).astype(np.float32)
    wnode = np.concatenate([Wda, Wsa, W_lin], axis=1)  # [C, 2AH + C]
    bias = np.zeros((128, 6), np.float32)
    bias[:, 5] = 0.499
    bias[: cfg.PH, 0] = bp1
    bias[: cfg.C, 1] = bp2
    bias[: cfg.AH, 2] = ba1
    bias[: cfg.C, 3] = ba2
    bias[:, 4] = -cfg.SHIFT

    packs, nchunk = _pack(x, pos, inputs["edge_index"], cfg)
    enc = _to_bf16_bits if cfg.w_bf16 else np.ascontiguousarray
    common = dict(Wnode=enc(wnode),
                  Wp1=np.ascontiguousarray(Wp1 / np.float32(127.0)), Wp2=enc(Wp2),
                  Wa2=enc(Wa2), bias=bias)
    common["Wa1p"] = enc(Wa1)
    in_maps = []
    for c, p in enumerate(packs):
        xsh = x[c * cfg.NSH:(c + 1) * cfg.NSH]           # [NSH, C]
        m = dict(common, xTsh=_to_bf16_bits(xsh.T), **p)
        in_maps.append(m)
    return in_maps, nchunk


def decode_y(y):
    if CFG.y_mode == "bf16":
        return y.view(BF16).astype(np.float32)
    if CFG.y_mode == "u8":
        return y.astype(np.float32) * np.float32(CFG.Y_SCALE)
    return y


def kernel(**inputs):
    cfg = CFG
    in_maps, nchunk = _build_inputs(inputs, cfg)
    nc = _build(cfg, nchunk)
    res = run_bass_kernel_spmd(nc, in_maps, list(range(cfg.M)))
    y = np.concatenate(
        [res.results[c]["y"][: cfg.NLOC] for c in range(cfg.M)], axis=0)
    return decode_y(y)



# revision 3
# speedup vs baseline: 3.3955x; 3.3955x over previous
"""Trainium2 Bass kernel for nn_ClusterEncoder (PointTransformerConv-style
GNN message passing), 8-core SPMD.

The metric regime is axon-tunnel transfer-bound (~21 MB/s host<->device),
so the kernel minimizes per-call host bytes:
  * x is NOT replicated: each core receives a transposed bf16 shard
    xT_sh = x[c*6250:(c+1)*6250].T  ([128, 6250], 1.6 MB/core) and the
    full xT is assembled on device with an AllGather collective.
  * Edge index tensors ship as uint16 (node ids < 50000) / int8
    (chunk-local dst slot), expanded to i32/f32 on device.
  * posdT ships as int8 (|posd|<1; the 1/127 dequant scale is folded
    into Wp1 host-side), weights ship as bf16 bit patterns.
  * y returns as u8 (scale 6/255; abs quant err ~0.012 vs 0.049 budget),
    decoded on host. bf16/f32 fallbacks behind Cfg.y_mode.

Compute strategy (edges sharded by destination node):
  * Host: sort edges by dst, split nodes into 8 equal contiguous ranges.
    Within a core, greedy-pack destination nodes into "chunks" of <=128
    nodes and <=CHUNK_E edges; pad each chunk's edge list to CHUNK_E.
  * Device, phase 0: AllGather xT shards -> xT_all [8*128, 6250]
    (block c = x[c*6250:(c+1)*6250].T).
  * Device, phase 1 (replicated): U = x @ (W_dst@Wa1),
    VH = x @ [W_src@Wa1 | W_lin] into DRAM; row-gatherable layouts.
    Uses xT_all blocks directly as matmul lhsT (no on-device transpose).
  * Device, phase 2 (per chunk of 16 x 128-edge tiles):
      - gather VH rows by src (768B/row) and U rows by dst,
      - gd = U[dst] - V[src]  (attn-layer-1 folded through node features),
      - pos MLP: t_p1 = relu(Wp1^T posd^T + bp1), delta = relu(Wp2^T t_p1 + bp2),
      - z1 = Wa1^T delta;  t_a = relu(z1 + gd^T + ba1),
      - logits = relu(Wa2^T t_a + ba2);  e = exp(logits - SHIFT)
        (softmax max-subtraction replaced by a constant shift -- exactly
        equivalent math since the shift cancels in e/sum(e); logits are
        relu-bounded so no overflow),
      - one-hot indicator per tile from local dst index (iota + is_equal),
      - segment-sum via matmul: acc[n, 0:128] += ind^T @ (e*(H[src]+delta))^T,
        acc[n, 128:256] += ind^T @ e^T   (numerator and normalizer together),
      - out = relu(NUM / (s + eps)); indirect-scatter rows to y (u8).
  * Softmax segments are core-local by construction (edges sharded by dst).
"""
import sys
from dataclasses import dataclass
from math import ceil

if "/opt/trn_rl_repo" not in sys.path:
    sys.path.insert(0, "/opt/trn_rl_repo")

import ml_dtypes
import numpy as np

import concourse.bass as bass
import concourse.mybir as mybir
import concourse.tile as tile
from concourse import bacc
from concourse.bass import IndirectOffsetOnAxis
from concourse.bass_utils import run_bass_kernel_spmd
from concourse.masks import make_identity

f32 = mybir.dt.float32
f32r = mybir.dt.float32r
bf16 = mybir.dt.bfloat16
i32 = mybir.dt.int32
u16 = mybir.dt.uint16
i8 = mybir.dt.int8
AF = mybir.ActivationFunctionType
ALU = mybir.AluOpType
BF16 = ml_dtypes.bfloat16


@dataclass
class Cfg:
    N: int = 50000
    C: int = 128
    PH: int = 64
    AH: int = 64
    DIM: int = 2
    M: int = 8            # cores
    T: int = 16           # 128-edge tiles per chunk
    TB: int = 4           # tiles per matmul block (block = 512 edges)
    SHIFT: float = 8.0
    EPS: float = 1e-12
    mm_dt: object = f32r  # matmul compute dtype (f32r: 1 cyc/row at free>=256)
    y_mode: str = "u8"    # "f32" | "bf16" | "u8" output encoding
    Y_SCALE: float = 6.0 / 255.0  # u8 quant step (y in [0, ~2.44]; 2.4x margin)
    pd_bf16_mm: bool = True   # feed posdT bf16 tile straight into matmul
    p1_bf16_mm: bool = False  # phase-1 matmul in bf16 (crashes exec unit; keep off)
    w_bf16: bool = True   # ship matmul weights as bf16

    @property
    def NLOC(self):
        return self.N // self.M

    @property
    def NSH(self):
        return self.N // self.M  # x rows per core shard

    @property
    def CHUNK_E(self):
        return self.T * 128

    @property
    def OUT_ROWS(self):
        return self.NLOC + 1  # +1 trash row for padded scatter lanes


CFG = Cfg()


def _to_bf16_bits(a):
    return np.ascontiguousarray(a.astype(BF16).view(np.uint16))


# ---------------------------------------------------------------- host pack
def _pack(x, pos, edge_index, cfg):
    """Sort/shard/chunk edges; returns per-core input dicts (minus weights)."""
    src = np.asarray(edge_index[0], np.int64)
    dst = np.asarray(edge_index[1], np.int64)
    order = np.argsort(dst, kind="stable")
    s_s = src[order]
    d_s = dst[order]
    posd = (pos[d_s] - pos[s_s]).astype(np.float32)  # [E, 2]

    NLOC = cfg.NLOC
    bounds = np.searchsorted(d_s, np.arange(cfg.M + 1) * NLOC)

    cores = []
    for c in range(cfg.M):
        lo, hi = bounds[c], bounds[c + 1]
        dloc = d_s[lo:hi] - c * NLOC
        deg = np.bincount(dloc, minlength=NLOC)
        nodes = np.nonzero(deg)[0]
        chunks = []  # (node_list, e0, e1) ; e relative to lo
        cur, cur_e, estart = [], 0, 0
        for n in nodes:
            dn = int(deg[n])
            assert dn <= cfg.CHUNK_E, f"degree {dn} exceeds chunk capacity"
            if len(cur) == 128 or cur_e + dn > cfg.CHUNK_E:
                chunks.append((cur, estart, estart + cur_e))
                estart += cur_e
                cur, cur_e = [], 0
            cur.append(int(n))
            cur_e += dn
        if cur:
            chunks.append((cur, estart, estart + cur_e))
        cores.append((lo, chunks, dloc))

    NCHUNK = max(len(ch) for _, ch, _ in cores) if cores else 1
    NCHUNK = max(NCHUNK, 1)

    in_maps = []
    for c in range(cfg.M):
        lo, chunks, dloc = cores[c]
        srcid = np.zeros((NCHUNK, 128, cfg.T), np.uint16)
        dstid = np.zeros((NCHUNK, 128, cfg.T), np.uint16)
        dstloc = np.full((NCHUNK, 128, cfg.T), -1, np.int8)
        posdT = np.zeros((NCHUNK, cfg.DIM, cfg.CHUNK_E), np.float32)
        outrow = np.full((NCHUNK, 128), cfg.NLOC, np.uint16)  # trash row
        for k, (nl, e0, e1) in enumerate(chunks):
            cnt = e1 - e0
            g0, g1 = lo + e0, lo + e1
            nla = np.asarray(nl, np.int64)
            loc = np.searchsorted(nla, dloc[e0:e1]).astype(np.int8)
            j = np.arange(cnt)
            t_idx = j >> 7
            lane = j & 127
            srcid[k, lane, t_idx] = s_s[g0:g1].astype(np.uint16)
            dstid[k, lane, t_idx] = d_s[g0:g1].astype(np.uint16)
            dstloc[k, lane, t_idx] = loc
            posdT[k, :, :cnt] = posd[g0:g1].T
            outrow[k, : len(nl)] = nla.astype(np.uint16)
        pq = np.clip(np.round(posdT * 127.0), -127, 127).astype(np.int8)
        in_maps.append(
            dict(srcid=srcid, dstid=dstid, dstloc=dstloc,
                 posdT=pq, outrow=outrow)
        )
    return in_maps, NCHUNK


# ---------------------------------------------------------------- program
def _build(cfg, nchunk):
    nc = bacc.Bacc(None, target_bir_lowering=False, num_devices=cfg.M)
    N, C, PH, AH, DIM = cfg.N, cfg.C, cfg.PH, cfg.AH, cfg.DIM
    NSH = cfg.NSH
    mdt = cfg.mm_dt

    w_dt = bf16 if cfg.w_bf16 else f32
    xts_d = nc.declare_dram_parameter("xTsh", [C, NSH], bf16, isOutput=False)
    wnode_d = nc.declare_dram_parameter("Wnode", [C, 2 * AH + C], w_dt, isOutput=False)
    wp1_d = nc.declare_dram_parameter("Wp1", [DIM, PH], f32, isOutput=False)
    wp2_d = nc.declare_dram_parameter("Wp2", [PH, C], w_dt, isOutput=False)
    wa1_d = nc.declare_dram_parameter("Wa1p", [C, AH], w_dt, isOutput=False)
    wa2_d = nc.declare_dram_parameter("Wa2", [AH, C], w_dt, isOutput=False)
    bias_d = nc.declare_dram_parameter("bias", [128, 6], f32, isOutput=False)
    src_d = nc.declare_dram_parameter("srcid", [nchunk, 128, cfg.T], u16, isOutput=False)
    dst_d = nc.declare_dram_parameter("dstid", [nchunk, 128, cfg.T], u16, isOutput=False)
    dl_d = nc.declare_dram_parameter("dstloc", [nchunk, 128, cfg.T], i8, isOutput=False)
    pd_d = nc.declare_dram_parameter("posdT", [nchunk, DIM, cfg.CHUNK_E], i8, isOutput=False)
    or_d = nc.declare_dram_parameter("outrow", [nchunk, 128], u16, isOutput=False)
    y_dt = {"f32": f32, "bf16": bf16, "u8": mybir.dt.uint8}[cfg.y_mode]
    y_d = nc.declare_dram_parameter("y", [cfg.OUT_ROWS, C], y_dt, isOutput=True)

    U_d = nc.dram_tensor("U", [N, AH], f32)          # x @ (W_dst@Wa1)
    VH_d = nc.dram_tensor("VH", [N, AH + C], f32)    # x @ [W_src@Wa1 | W_lin]
    xg_in = nc.dram_tensor("xg_in", [C, NSH], bf16)  # collective input bounce
    xT_all = nc.dram_tensor("xT_all", [cfg.M * C, NSH], bf16)

    NB = cfg.T // cfg.TB  # blocks per chunk
    BLK = cfg.TB * 128

    with tile.TileContext(nc) as tc:
        with tc.tile_pool(name="const", bufs=1) as cp:
            wnode_s = cp.tile([C, 2 * AH + C], w_dt)
            nc.sync.dma_start(out=wnode_s[:], in_=wnode_d[:, :])
            wp1_s = cp.tile([DIM, PH], f32)
            nc.sync.dma_start(out=wp1_s[:], in_=wp1_d[:, :])
            wp2_s = cp.tile([PH, C], w_dt)
            nc.sync.dma_start(out=wp2_s[:], in_=wp2_d[:, :])
            wa2_s = cp.tile([AH, C], w_dt)
            nc.sync.dma_start(out=wa2_s[:], in_=wa2_d[:, :])
            bias_s = cp.tile([128, 6], f32)
            nc.sync.dma_start(out=bias_s[:], in_=bias_d[:, :])
            ident_s = cp.tile([128, 128], f32)
            make_identity(nc, ident_s[:])
            iota_i = cp.tile([128, 128], i32)
            nc.gpsimd.iota(iota_i[:], pattern=[[1, 128]], base=0, channel_multiplier=0)
            iota_s = cp.tile([128, 128], f32)
            nc.vector.tensor_copy(iota_s[:], iota_i[:])
            wa1_s = cp.tile([C, AH], w_dt)
            nc.sync.dma_start(out=wa1_s[:], in_=wa1_d[:, :])

            # phase-1 weights in bf16 (match bf16 x operand)
            p1_dt = bf16 if cfg.p1_bf16_mm else mdt
            wnode_b = cp.tile([C, 2 * AH + C], p1_dt)
            nc.vector.tensor_copy(wnode_b[:], wnode_s[:])
            # pos-MLP layer-1 weights: match the pd operand dtype
            pd_mm_dt = bf16 if cfg.pd_bf16_mm else mdt
            wp1_b = cp.tile([DIM, PH], pd_mm_dt)
            nc.vector.tensor_copy(wp1_b[:], wp1_s[:])

            # fp32r matmul operands must be produced rounded-to-f32r: make
            # rounded copies of the stationary weights once.
            if mdt is f32r:
                wp2_m = cp.tile([PH, C], f32r)
                nc.vector.tensor_copy(wp2_m[:], wp2_s[:])
                wa1_m = cp.tile([C, AH], f32r)
                nc.vector.tensor_copy(wa1_m[:], wa1_s[:])
                wa2_m = cp.tile([AH, C], f32r)
                nc.vector.tensor_copy(wa2_m[:], wa2_s[:])
            else:
                wp2_m, wa1_m, wa2_m = wp2_s, wa1_s, wa2_s

            # ---------------- phase 0: AllGather x shards ----------------
            nc.gpsimd.dma_start(out=xg_in[:, :], in_=xts_d[:, :])
            nc.gpsimd.collective_compute(
                "AllGather",
                mybir.AluOpType.bypass,
                replica_groups=[list(range(cfg.M))],
                ins=[xg_in[:, :]],
                outs=[xT_all[:, :]],
            )

            # ---------------- phase 1: node features U / VH ----------------
            # xT_all block c8 = x[c8*NSH:(c8+1)*NSH].T  -> use as matmul lhsT
            with tc.tile_pool(name="p1", bufs=3) as p1, \
                 tc.tile_pool(name="p1ps", bufs=2, space="PSUM") as p1ps:
                ncol = ceil(NSH / 128)
                for c8 in range(cfg.M):
                    for t in range(ncol):
                        j0 = t * 128
                        rows = min(128, NSH - j0)
                        r0 = c8 * NSH + j0
                        xt = p1.tile([C, 128], bf16, tag="xt")
                        nc.sync.dma_start(
                            out=xt[:, :rows],
                            in_=xT_all[c8 * C:(c8 + 1) * C, j0:j0 + rows])
                        if cfg.p1_bf16_mm:
                            xt_m = xt
                        else:
                            xt_m = p1.tile([C, 128], mdt, tag="xtm")
                            nc.vector.tensor_copy(xt_m[:, :rows], xt[:, :rows])
                        uvh_p = p1ps.tile([128, 2 * AH + C], f32, tag="uvh")
                        nc.tensor.matmul(uvh_p[:rows, :], lhsT=xt_m[:, :rows],
                                         rhs=wnode_b[:], start=True, stop=True)
                        uvh_s = p1.tile([128, 2 * AH + C], f32, tag="uvhs")
                        nc.scalar.activation(uvh_s[:rows, :], uvh_p[:rows, :], AF.Copy)
                        nc.sync.dma_start(out=U_d[r0:r0 + rows, :], in_=uvh_s[:rows, 0:AH])
                        nc.sync.dma_start(out=VH_d[r0:r0 + rows, :], in_=uvh_s[:rows, AH:])

            # ---------------- phase 2: edges ----------------
            with tc.tile_pool(name="eb", bufs=3) as eb, \
                 tc.tile_pool(name="ebg", bufs=3) as ebg, \
                 tc.tile_pool(name="ps_acc", bufs=2, space="PSUM") as ps_acc, \
                 tc.tile_pool(name="ps_b", bufs=1, space="PSUM") as ps_b, \
                 tc.tile_pool(name="ps_c", bufs=1, space="PSUM") as ps_c, \
                 tc.tile_pool(name="ps_m", bufs=1, space="PSUM") as ps_m, \
                 tc.tile_pool(name="ps_n", bufs=1, space="PSUM") as ps_n, \
                 tc.tile_pool(name="ps_t", bufs=2, space="PSUM") as ps_t:
                for k in range(nchunk):
                    src16 = eb.tile([128, cfg.T], u16, tag="src16")
                    nc.sync.dma_start(out=src16[:], in_=src_d[k, :, :])
                    src_s = eb.tile([128, cfg.T], i32, tag="src")
                    nc.vector.tensor_copy(src_s[:], src16[:])
                    dst16 = eb.tile([128, cfg.T], u16, tag="dst16")
                    nc.sync.dma_start(out=dst16[:], in_=dst_d[k, :, :])
                    dst_s = eb.tile([128, cfg.T], i32, tag="dst")
                    nc.vector.tensor_copy(dst_s[:], dst16[:])
                    dl8 = eb.tile([128, cfg.T], i8, tag="dl8")
                    nc.sync.dma_start(out=dl8[:], in_=dl_d[k, :, :])
                    dl_s = eb.tile([128, cfg.T], f32, tag="dl")
                    nc.vector.tensor_copy(dl_s[:], dl8[:])
                    pd_s = eb.tile([DIM, cfg.CHUNK_E], i8, tag="pd")
                    nc.sync.dma_start(out=pd_s[:], in_=pd_d[k, :, :])
                    pd_mm = bf16 if cfg.pd_bf16_mm else mdt
                    pd_m = eb.tile([DIM, cfg.CHUNK_E], pd_mm, tag="pdm")
                    nc.vector.tensor_copy(pd_m[:], pd_s[:])
                    or16 = eb.tile([128, 1], u16, tag="or16")
                    nc.sync.dma_start(out=or16[:], in_=or_d[k, :, None])
                    or_s = eb.tile([128, 1], i32, tag="or")
                    nc.vector.tensor_copy(or_s[:], or16[:])

                    acc_p = ps_acc.tile([128, 2 * C], f32, tag="acc")

                    for b in range(NB):
                        esl = slice(b * BLK, (b + 1) * BLK)
                        # gathers for this block, one [128,1]-offset DMA per tile
                        vhgs, ugs = [], []
                        for tt in range(cfg.TB):
                            ti = b * cfg.TB + tt
                            vhg_t = ebg.tile([128, AH + C], f32, tag=f"vhg{tt}")
                            nc.gpsimd.indirect_dma_start(
                                out=vhg_t[:], out_offset=None, in_=VH_d[:],
                                in_offset=IndirectOffsetOnAxis(
                                    ap=src_s[:, ti:ti + 1], axis=0))
                            vhgs.append(vhg_t)
                            ug_t = ebg.tile([128, AH], f32, tag=f"ug{tt}")
                            nc.gpsimd.indirect_dma_start(
                                out=ug_t[:], out_offset=None, in_=U_d[:],
                                in_offset=IndirectOffsetOnAxis(
                                    ap=dst_s[:, ti:ti + 1], axis=0))
                            ugs.append(ug_t)

                        # pos MLP
                        tp1_p = ps_m.tile([PH, BLK], f32, tag="tp1")
                        nc.tensor.matmul(tp1_p[:], lhsT=wp1_b[:],
                                         rhs=pd_m[:, esl], start=True, stop=True)
                        tp1_s = eb.tile([PH, BLK], mdt, tag="tp1s")
                        nc.scalar.activation(tp1_s[:], tp1_p[:], AF.Relu, bias=bias_s[0:PH, 0:1])
                        del_p = ps_b.tile([C, BLK], f32, tag="delp")
                        nc.tensor.matmul(del_p[:], lhsT=wp2_m[:],
                                         rhs=tp1_s[:], start=True, stop=True)
                        del_s = eb.tile([C, BLK], f32, tag="dels")
                        nc.scalar.activation(del_s[:], del_p[:], AF.Relu, bias=bias_s[:, 1:2])
                        if mdt is f32r:
                            del_m = eb.tile([C, BLK], f32r, tag="delm")
                            nc.scalar.activation(del_m[:], del_p[:], AF.Relu, bias=bias_s[:, 1:2])
                        else:
                            del_m = del_s

                        # attn layer 1: z1 = Wa1^T delta ; t_a = relu(z1 + gd^T + ba1)
                        z1_p = ps_n.tile([AH, BLK], f32, tag="z1")
                        nc.tensor.matmul(z1_p[:], lhsT=wa1_m[:],
                                         rhs=del_m[:], start=True, stop=True)
                        tsum_s = eb.tile([AH, BLK], f32, tag="tsum")
                        for tt in range(cfg.TB):
                            gd_s = eb.tile([128, AH], f32, tag="gd")
                            nc.vector.tensor_tensor(gd_s[:], ugs[tt][:], vhgs[tt][:, 0:AH],
                                                    op=ALU.subtract)
                            gdT_p = ps_t.tile([128, 128], f32, tag="tr")
                            nc.tensor.transpose(gdT_p[:AH, :], gd_s[:], ident_s[:])
                            gdT_s = eb.tile([AH, 128], f32, tag="gdT")
                            nc.scalar.activation(gdT_s[:], gdT_p[:AH, :], AF.Copy)
                            csl = slice(tt * 128, (tt + 1) * 128)
                            nc.vector.tensor_tensor(tsum_s[:, csl], z1_p[:, csl],
                                                    gdT_s[:], op=ALU.add)
                        ta_s = eb.tile([AH, BLK], mdt, tag="ta")
                        nc.scalar.activation(ta_s[:], tsum_s[:], AF.Relu, bias=bias_s[0:AH, 2:3])

                        # attn layer 2 + exp
                        al_p = ps_c.tile([C, BLK], f32, tag="al")
                        nc.tensor.matmul(al_p[:], lhsT=wa2_m[:],
                                         rhs=ta_s[:], start=True, stop=True)
                        ar_s = eb.tile([C, BLK], f32, tag="ar")
                        nc.scalar.activation(ar_s[:], al_p[:], AF.Relu, bias=bias_s[:, 3:4])
                        e_s = eb.tile([C, BLK], f32, tag="e")
                        nc.scalar.activation(e_s[:], ar_s[:], AF.Exp, bias=bias_s[:, 4:5])
                        ew2_s = eb.tile([C, BLK], f32, tag="ew2")
                        nc.vector.tensor_tensor(ew2_s[:], e_s[:], del_s[:], op=ALU.mult)
                        del del_s  # f32 copy only feeds ew2

                        # per-tile: transpose, assemble [ew | e]^T, indicator, seg-matmul
                        for tt in range(cfg.TB):
                            ti = b * cfg.TB + tt
                            csl = slice(tt * 128, (tt + 1) * 128)
                            eT_p = ps_t.tile([128, 128], f32, tag="tr")
                            nc.tensor.transpose(eT_p[:], e_s[:, csl], ident_s[:])
                            ew2T_p = ps_t.tile([128, 128], f32, tag="tr")
                            nc.tensor.transpose(ew2T_p[:], ew2_s[:, csl], ident_s[:])
                            ewe_s = eb.tile([128, 2 * C], mdt, tag="ewe")
                            nc.vector.tensor_copy(ewe_s[:, C:], eT_p[:])
                            tmp_s = eb.tile([128, C], f32, tag="tmp")
                            nc.vector.tensor_tensor(tmp_s[:], eT_p[:], vhgs[tt][:, AH:],
                                                    op=ALU.mult)
                            nc.vector.tensor_tensor(ewe_s[:, 0:C], tmp_s[:], ew2T_p[:],
                                                    op=ALU.add)
                            ind_s = eb.tile([128, 128], mdt, tag="ind")
                            nc.vector.tensor_scalar(ind_s[:], iota_s[:], dl_s[:, ti:ti + 1],
                                                    None, op0=ALU.is_equal)
                            nc.tensor.matmul(acc_p[:], lhsT=ind_s[:],
                                             rhs=ewe_s[:],
                                             start=(ti == 0), stop=(ti == cfg.T - 1))

                    # finalize chunk
                    sp_s = eb.tile([128, C], f32, tag="sp")
                    nc.vector.tensor_scalar_add(sp_s[:], acc_p[:, C:], cfg.EPS)
                    rp_s = eb.tile([128, C], f32, tag="rp")
                    nc.vector.reciprocal(rp_s[:], sp_s[:])
                    o_s = eb.tile([128, C], f32, tag="o")
                    nc.vector.tensor_tensor(o_s[:], acc_p[:, 0:C], rp_s[:], op=ALU.mult)
                    o2_s = eb.tile([128, C], y_dt, tag="o2")
                    if cfg.y_mode == "u8":
                        # f32->u8 conversion rounds to nearest
                        nc.scalar.activation(o2_s[:], o_s[:], AF.Relu,
                                             scale=1.0 / cfg.Y_SCALE)
                    else:
                        nc.scalar.activation(o2_s[:], o_s[:], AF.Relu)
                    nc.gpsimd.indirect_dma_start(
                        out=y_d[:], out_offset=IndirectOffsetOnAxis(ap=or_s[:, :1], axis=0),
                        in_=o2_s[:], in_offset=None)
    nc.finalize()
    return nc


def _build_inputs(inputs, cfg):
    x = np.ascontiguousarray(np.asarray(inputs["x"], np.float32))
    pos = np.ascontiguousarray(np.asarray(inputs["pos"], np.float32))
    W_lin = np.asarray(inputs["W_lin"], np.float32)
    W_src = np.asarray(inputs["W_src"], np.float32)
    W_dst = np.asarray(inputs["W_dst"], np.float32)
    Wp1 = np.asarray(inputs["Wp1"], np.float32)
    bp1 = np.asarray(inputs["bp1"], np.float32)
    Wp2 = np.asarray(inputs["Wp2"], np.float32)
    bp2 = np.asarray(inputs["bp2"], np.float32)
    Wa1 = np.asarray(inputs["Wa1"], np.float32)
    ba1 = np.asarray(inputs["ba1"], np.float32)
    Wa2 = np.asarray(inputs["Wa2"], np.float32)
    ba2 = np.asarray(inputs["ba2"], np.float32)

    Wda = (W_dst @ Wa1).astype(np.float32)   # [C, AH]
    Wsa = (W_src @ Wa1).astype(np.float32)
    wnode = np.concatenate([Wda, Wsa, W_lin], axis=1)  # [C, 2AH + C]
    bias = np.zeros((128, 6), np.float32)
    bias[:, 5] = 0.499
    bias[: cfg.PH, 0] = bp1
    bias[: cfg.C, 1] = bp2
    bias[: cfg.AH, 2] = ba1
    bias[: cfg.C, 3] = ba2
    bias[:, 4] = -cfg.SHIFT

    packs, nchunk = _pack(x, pos, inputs["edge_index"], cfg)
    enc = _to_bf16_bits if cfg.w_bf16 else np.ascontiguousarray
    common = dict(Wnode=enc(wnode),
                  Wp1=np.ascontiguousarray(Wp1 / np.float32(127.0)), Wp2=enc(Wp2),
                  Wa2=enc(Wa2), bias=bias)
    common["Wa1p"] = enc(Wa1)
    in_maps = []
    for c, p in enumerate(packs):
        xsh = x[c * cfg.NSH:(c + 1) * cfg.NSH]           # [NSH, C]
        m = dict(common, xTsh=_to_bf16_bits(xsh.T), **p)
        in_maps.append(m)
    return in_maps, nchunk


def decode_y(y):
    if CFG.y_mode == "bf16":
        return y.view(BF16).astype(np.float32)
    if CFG.y_mode == "u8":
        return y.astype(np.float32) * np.float32(CFG.Y_SCALE)
    return y


# ---------------------------------------------------------------- runner
# Mirror of bass2jax.run_bass_via_pjrt, with two wall-clock fixes for the
# per-call path:
#   * the jitted shard_map executable is built ONCE and cached (the stock
#     helper re-jits a fresh closure every call -> ~1.3s of retrace/XLA
#     re-lowering per call),
#   * the zero-initialized ExternalOutput buffers are device-resident and
#     reused (not re-uploaded per call; the custom call copies them into
#     the result buffer device-side).
# Every call still ships all in_map bytes host->device, executes, and
# fetches the outputs back to numpy.
_RUNNER = {}


def _make_runner(nc, n_cores):
    import jax
    from jax.sharding import Mesh, PartitionSpec, NamedSharding
    from jax.experimental.shard_map import shard_map
    from concourse.bass2jax import (
        _bass_exec_p, partition_id_tensor, install_neuronx_cc_hook)

    install_neuronx_cc_hook()
    assert not nc.dbg_callbacks
    partition_name = (
        nc.partition_id_tensor.name if nc.partition_id_tensor else None)
    in_names, out_names, out_avals, zero_outs = [], [], [], []
    for alloc in nc.m.functions[0].allocations:
        if not isinstance(alloc, mybir.MemoryLocationSet):
            continue
        name = alloc.memorylocations[0].name
        if alloc.kind == "ExternalInput":
            if name != partition_name and name != (
                    nc.dbg_addr.name if nc.dbg_addr is not None else None):
                in_names.append(name)
        elif alloc.kind == "ExternalOutput":
            shape = tuple(alloc.tensor_shape)
            dtype = mybir.dt.np(alloc.dtype)
            out_avals.append(jax.core.ShapedArray(shape, dtype))
            zero_outs.append(np.zeros(shape, dtype))
            out_names.append(name)
    n_params = len(in_names)
    in_names_all = list(in_names) + out_names
    if nc.dbg_addr is not None:
        in_names_all.append(nc.dbg_addr.name)
    if partition_name is not None:
        in_names_all.append(partition_name)

    def _body(*args):
        operands = list(args)
        if nc.dbg_addr is not None:
            operands.append(jax.numpy.zeros((1, 2), jax.numpy.uint32))
        if partition_name is not None:
            operands.append(partition_id_tensor())
        return tuple(_bass_exec_p.bind(
            *operands, out_avals=tuple(out_avals),
            in_names=tuple(in_names_all), out_names=tuple(out_names),
            lowering_input_output_aliases=(),
            sim_require_finite=True, sim_require_nnan=True, nc=nc))

    devices = jax.devices()[:n_cores]
    mesh = Mesh(np.asarray(devices), ("core",))
    nsh = NamedSharding(mesh, PartitionSpec("core"))
    n_outs = len(out_avals)
    sharded = jax.jit(
        shard_map(_body, mesh=mesh,
                  in_specs=(PartitionSpec("core"),) * (n_params + n_outs),
                  out_specs=(PartitionSpec("core"),) * n_outs,
                  check_rep=False),
        keep_unused=True)
    dev_zeros = [
        jax.device_put(
            np.zeros((n_cores * z.shape[0], *z.shape[1:]), z.dtype), nsh)
        for z in zero_outs]

    def run(in_maps):
        concat_in = [
            np.concatenate([np.asarray(m[nm]) for m in in_maps], axis=0)
            for nm in in_names]
        out_arrs = sharded(*concat_in, *dev_zeros)
        return [
            {name: np.asarray(out_arrs[i]).reshape(
                n_cores, *out_avals[i].shape)[c]
             for i, name in enumerate(out_names)}
            for c in range(n_cores)]

    return run


def run_cached(nc, in_maps, n_cores):
    key = id(nc)
    if key not in _RUNNER:
        _RUNNER[key] = _make_runner(nc, n_cores)
    return _RUNNER[key](in_maps)


def kernel(**inputs):
    cfg = CFG
    in_maps, nchunk = _build_inputs(inputs, cfg)
    nc = _build(cfg, nchunk)
    results = run_cached(nc, in_maps, cfg.M)
    y = np.concatenate(
        [results[c]["y"][: cfg.NLOC] for c in range(cfg.M)], axis=0)
    return decode_y(y)



# revision 7
# speedup vs baseline: 4.9806x; 1.4668x over previous
"""Trainium2 Bass kernel for nn_ClusterEncoder (PointTransformerConv-style
GNN message passing), 8-core SPMD.

The metric regime is axon-tunnel transfer-bound (tens of MB/s host<->device
plus ~80ms fixed dispatch), so the kernel minimizes per-call host bytes and
per-call Python/XLA overhead:
  * x ships as int8 with a per-node f32 scale (xT shard [128, 6250] i8 +
    scale [6250,1] f32 per core); full tables are assembled on device with
    AllGather collectives and the scale is applied post-matmul (U/V/H rows
    scale linearly in x_n).
  * pos ships as u16 codes (pos*65535), sharded + AllGathered; the 1/65535
    dequant is folded into Wp1 host-side.
  * No per-edge dst metadata: edges are sorted by dst and grouped into
    <=128-node chunks; the per-chunk edge->node indicator matrix is built
    on device from cumulative degrees (u16 [128,2] per chunk) and an iota,
    replacing the dstid/dstloc uploads. posd is never shipped: the pos MLP
    layer-1 (pos @ Wp1) is computed per node in phase 1 and distributed to
    edges through the same indicator matmul (it is linear, so
    P1[dst]-P1[src] can replace Wp1^T posd).
  * y returns as u8 (scale 2.75/255), decoded on host.
  * run_cached keeps ONE jitted shard_map executable alive across calls and
    keeps the zero-filled output donation buffers device-resident, so a
    steady-state call is: concat numpy inputs -> h2d -> exec -> d2h.

Compute strategy (edges sharded by destination node):
  * Host: sort edges by dst, split nodes into 8 equal contiguous ranges.
    Greedy-pack destination nodes into chunks of <=128 nodes / <=CHUNK_E
    edges; ship per-chunk src ids (u16), local out rows (u16), and
    cumulative-degree pairs (u16).
  * Device, phase 0: AllGather x int8 shards, x scales, posT u16 shards.
  * Device, phase 1 (replicated): per 128-node tile
      uvhp = [x@Wda | x@Wsa | x@Wlin | pos@Wp1'] (two matmuls into one
      PSUM tile), scale the x part per node, write tables
      G = [U | P1]  [N+1, 128] and S = [V | H | P1]  [N, 256] to DRAM.
  * Device, phase 2 (per chunk of 16 x 128-edge tiles):
      - gather G rows once per chunk by outrow (dst nodes), gather S rows
        per edge by src id,
      - indicator indT[n,e] = (e>=cum[n]) - (e>=cum[n+1]) from iota,
      - dst-side distribute: Du = G_U^T @ indT, Dp = G_P1^T @ indT,
      - t_p1 = relu(Dp - P1s^T + bp1);  delta = relu(Wp2^T t_p1 + bp2),
      - t_a = relu(Wa1^T delta + (Du - V^T) + ba1),
      - logits = relu(Wa2^T t_a + ba2);  e = exp(logits - SHIFT)
        (constant shift cancels in e/sum(e); logits are relu-bounded),
      - seg-sum via matmul: acc[n,0:128] += ind^T @ (e*(H[src]+delta))^T,
        acc[n,128:256] += ind^T @ e^T,
      - out = relu(NUM / (s + eps)); indirect-scatter rows to y (u8).
  * Softmax segments are core-local by construction (edges sharded by dst).
"""
import sys
from dataclasses import dataclass
from math import ceil

if "/opt/trn_rl_repo" not in sys.path:
    sys.path.insert(0, "/opt/trn_rl_repo")

import ml_dtypes
import numpy as np

import concourse.bass as bass
import concourse.mybir as mybir
import concourse.tile as tile
from concourse import bacc
from concourse.bass import IndirectOffsetOnAxis
from concourse.bass_utils import run_bass_kernel_spmd
from concourse.masks import make_identity

f32 = mybir.dt.float32
f32r = mybir.dt.float32r
bf16 = mybir.dt.bfloat16
i32 = mybir.dt.int32
u16 = mybir.dt.uint16
i8 = mybir.dt.int8
AF = mybir.ActivationFunctionType
ALU = mybir.AluOpType
BF16 = ml_dtypes.bfloat16


@dataclass
class Cfg:
    N: int = 50000
    C: int = 128
    PH: int = 64
    AH: int = 64
    DIM: int = 2
    M: int = 8            # cores
    T: int = 16           # 128-edge tiles per chunk
    TB: int = 4           # tiles per matmul block (block = 512 edges)
    SHIFT: float = 8.0
    EPS: float = 1e-12
    mm_dt: object = f32r  # matmul compute dtype (f32r: 1 cyc/row at free>=256)
    y_mode: str = "u8"    # "f32" | "u8" output encoding
    Y_SCALE: float = 2.75 / 255.0  # u8 quant step (y in [0, ~2.44])
    w_bf16: bool = True   # ship matmul weights as bf16

    @property
    def NLOC(self):
        return self.N // self.M

    @property
    def NSH(self):
        return self.N // self.M  # x rows per core shard

    @property
    def CHUNK_E(self):
        return self.T * 128

    @property
    def OUT_ROWS(self):
        return self.NLOC + 1  # +1 trash row for padded scatter lanes


CFG = Cfg()


def _to_bf16_bits(a):
    return np.ascontiguousarray(a.astype(BF16).view(np.uint16))


# ---------------------------------------------------------------- host pack
def _pack(x, pos, edge_index, cfg):
    """Sort/shard/chunk edges; returns per-core input dicts (minus weights)."""
    src = np.asarray(edge_index[0], np.int64)
    dst = np.asarray(edge_index[1], np.int64)
    order = np.argsort(dst, kind="stable")
    s_s = src[order]
    d_s = dst[order]

    NLOC = cfg.NLOC
    bounds = np.searchsorted(d_s, np.arange(cfg.M + 1) * NLOC)

    cores = []
    for c in range(cfg.M):
        lo, hi = bounds[c], bounds[c + 1]
        dloc = d_s[lo:hi] - c * NLOC
        deg = np.bincount(dloc, minlength=NLOC)
        nodes = np.nonzero(deg)[0]
        chunks = []  # (node_list, e0, e1) ; e relative to lo
        cur, cur_e, estart = [], 0, 0
        for n in nodes:
            dn = int(deg[n])
            assert dn <= cfg.CHUNK_E, f"degree {dn} exceeds chunk capacity"
            if len(cur) == 128 or cur_e + dn > cfg.CHUNK_E:
                chunks.append((cur, estart, estart + cur_e))
                estart += cur_e
                cur, cur_e = [], 0
            cur.append(int(n))
            cur_e += dn
        if cur:
            chunks.append((cur, estart, estart + cur_e))
        cores.append((lo, chunks, deg))

    NCHUNK = max(max(len(ch) for _, ch, _ in cores), 1)

    in_maps = []
    for c in range(cfg.M):
        lo, chunks, deg = cores[c]
        srcid = np.zeros((NCHUNK, 128, cfg.T), np.uint16)
        cum2 = np.zeros((NCHUNK, 128, 2), np.uint16)
        outrow = np.full((NCHUNK, 128), cfg.NLOC, np.uint16)  # trash row
        for k, (nl, e0, e1) in enumerate(chunks):
            cnt = e1 - e0
            g0, g1 = lo + e0, lo + e1
            nla = np.asarray(nl, np.int64)
            j = np.arange(cnt)
            srcid[k, j & 127, j >> 7] = s_s[g0:g1].astype(np.uint16)
            cums = np.concatenate([[0], np.cumsum(deg[nla])])
            cum2[k, :, :] = cnt
            cum2[k, : len(nl), 0] = cums[:-1]
            cum2[k, : len(nl), 1] = cums[1:]
            outrow[k, : len(nl)] = nla.astype(np.uint16)
        in_maps.append(dict(srcid=srcid, cum2=cum2, outrow=outrow))
    return in_maps, NCHUNK


# ---------------------------------------------------------------- program
def _build(cfg, nchunk):
    nc = bacc.Bacc(None, target_bir_lowering=False, num_devices=cfg.M)
    N, C, PH, AH, DIM = cfg.N, cfg.C, cfg.PH, cfg.AH, cfg.DIM
    NSH = cfg.NSH
    mdt = cfg.mm_dt

    w_dt = bf16 if cfg.w_bf16 else f32
    xq_d = nc.declare_dram_parameter("xqsh", [C, NSH], i8, isOutput=False)
    xsc_d = nc.declare_dram_parameter("xsc", [NSH, 1], f32, isOutput=False)
    pos_d = nc.declare_dram_parameter("posT", [DIM, NSH], u16, isOutput=False)
    wnode_d = nc.declare_dram_parameter("Wnode", [C, 2 * AH + C], w_dt, isOutput=False)
    wp1_d = nc.declare_dram_parameter("Wp1", [DIM, PH], f32, isOutput=False)
    wp2_d = nc.declare_dram_parameter("Wp2", [PH, C], w_dt, isOutput=False)
    wa1_d = nc.declare_dram_parameter("Wa1p", [C, AH], w_dt, isOutput=False)
    wa2_d = nc.declare_dram_parameter("Wa2", [AH, C], w_dt, isOutput=False)
    bias_d = nc.declare_dram_parameter("bias", [128, 5], f32, isOutput=False)
    src_d = nc.declare_dram_parameter("srcid", [nchunk, 128, cfg.T], u16, isOutput=False)
    cum_d = nc.declare_dram_parameter("cum2", [nchunk, 128, 2], u16, isOutput=False)
    or_d = nc.declare_dram_parameter("outrow", [nchunk, 128], u16, isOutput=False)
    base_d = nc.declare_dram_parameter("basec", [128, 1], i32, isOutput=False)
    y_dt = {"f32": f32, "u8": mybir.dt.uint8}[cfg.y_mode]
    y_d = nc.declare_dram_parameter("y", [cfg.OUT_ROWS, C], y_dt, isOutput=True)

    # node tables: G = [U | P1] (dst side), S = [V | H | P1] (src side)
    G_d = nc.dram_tensor("G", [N + 1, AH + PH], f32)
    S_d = nc.dram_tensor("S", [N, AH + C + PH], f32)
    xg_in = nc.dram_tensor("xg_in", [C, NSH], i8)
    xq_all = nc.dram_tensor("xq_all", [cfg.M * C, NSH], i8)
    sc_in = nc.dram_tensor("sc_in", [NSH, 1], f32)
    sc_all = nc.dram_tensor("sc_all", [cfg.M * NSH, 1], f32)
    pg_in = nc.dram_tensor("pg_in", [DIM, NSH], u16)
    pos_all = nc.dram_tensor("pos_all", [cfg.M * DIM, NSH], u16)

    NB = cfg.T // cfg.TB  # blocks per chunk
    BLK = cfg.TB * 128
    NCOL = 2 * AH + C  # 256: U | V | H psum columns
    NPS = NCOL + PH    # 320: + P1

    with tile.TileContext(nc) as tc:
        with tc.tile_pool(name="const", bufs=1) as cp:
            wnode_s = cp.tile([C, NCOL], w_dt)
            nc.sync.dma_start(out=wnode_s[:], in_=wnode_d[:, :])
            wp1_s = cp.tile([DIM, PH], f32)
            nc.sync.dma_start(out=wp1_s[:], in_=wp1_d[:, :])
            wp2_s = cp.tile([PH, C], w_dt)
            nc.sync.dma_start(out=wp2_s[:], in_=wp2_d[:, :])
            wa1_s = cp.tile([C, AH], w_dt)
            nc.sync.dma_start(out=wa1_s[:], in_=wa1_d[:, :])
            wa2_s = cp.tile([AH, C], w_dt)
            nc.sync.dma_start(out=wa2_s[:], in_=wa2_d[:, :])
            bias_s = cp.tile([128, 5], f32)
            nc.sync.dma_start(out=bias_s[:], in_=bias_d[:, :])
            base_s = cp.tile([128, 1], i32)
            nc.sync.dma_start(out=base_s[:], in_=base_d[:, :])
            ident_s = cp.tile([128, 128], f32)
            make_identity(nc, ident_s[:])
            ident_r = cp.tile([128, 128], mdt)
            nc.vector.tensor_copy(ident_r[:], ident_s[:])
            iot_i = cp.tile([128, cfg.CHUNK_E], i32)
            nc.gpsimd.iota(iot_i[:], pattern=[[1, cfg.CHUNK_E]], base=0,
                           channel_multiplier=0)
            iot_f = cp.tile([128, cfg.CHUNK_E], f32)
            nc.vector.tensor_copy(iot_f[:], iot_i[:])

            # rounded-to-f32r stationary weights
            wnode_m = cp.tile([C, NCOL], mdt)
            nc.vector.tensor_copy(wnode_m[:], wnode_s[:])
            wp1_m = cp.tile([DIM, PH], mdt)
            nc.vector.tensor_copy(wp1_m[:], wp1_s[:])
            wp2_m = cp.tile([PH, C], mdt)
            nc.vector.tensor_copy(wp2_m[:], wp2_s[:])
            wa1_m = cp.tile([C, AH], mdt)
            nc.vector.tensor_copy(wa1_m[:], wa1_s[:])
            wa2_m = cp.tile([AH, C], mdt)
            nc.vector.tensor_copy(wa2_m[:], wa2_s[:])

            # ------------- phase 0: AllGather x / scales / pos -------------
            groups = [list(range(cfg.M))]
            nc.gpsimd.dma_start(out=xg_in[:, :], in_=xq_d[:, :])
            nc.gpsimd.collective_compute(
                "AllGather", mybir.AluOpType.bypass, replica_groups=groups,
                ins=[xg_in[:, :]], outs=[xq_all[:, :]])
            nc.gpsimd.dma_start(out=sc_in[:, :], in_=xsc_d[:, :])
            nc.gpsimd.collective_compute(
                "AllGather", mybir.AluOpType.bypass, replica_groups=groups,
                ins=[sc_in[:, :]], outs=[sc_all[:, :]])
            nc.gpsimd.dma_start(out=pg_in[:, :], in_=pos_d[:, :])
            nc.gpsimd.collective_compute(
                "AllGather", mybir.AluOpType.bypass, replica_groups=groups,
                ins=[pg_in[:, :]], outs=[pos_all[:, :]])

            # ------------- phase 1: node tables G / S -------------
            with tc.tile_pool(name="p1", bufs=3) as p1, \
                 tc.tile_pool(name="p1ps", bufs=2, space="PSUM") as p1ps:
                # zero the trash row N of G (gathered by padded outrow slots)
                zrow = p1.tile([1, AH + PH], f32, tag="zrow")
                nc.vector.memset(zrow[:], 0.0)
                nc.sync.dma_start(out=G_d[N:N + 1, :], in_=zrow[:])
                ncol = ceil(NSH / 128)
                for c8 in range(cfg.M):
                    for t in range(ncol):
                        j0 = t * 128
                        rows = min(128, NSH - j0)
                        r0 = c8 * NSH + j0
                        xt = p1.tile([C, 128], i8, tag="xt")
                        nc.sync.dma_start(
                            out=xt[:, :rows],
                            in_=xq_all[c8 * C:(c8 + 1) * C, j0:j0 + rows])
                        xt_m = p1.tile([C, 128], mdt, tag="xtm")
                        nc.vector.tensor_copy(xt_m[:, :rows], xt[:, :rows])
                        pt = p1.tile([DIM, 128], u16, tag="pt")
                        nc.sync.dma_start(
                            out=pt[:, :rows],
                            in_=pos_all[c8 * DIM:(c8 + 1) * DIM, j0:j0 + rows])
                        pt_m = p1.tile([DIM, 128], mdt, tag="ptm")
                        nc.vector.tensor_copy(pt_m[:, :rows], pt[:, :rows])
                        sc_t = p1.tile([128, 1], f32, tag="sc")
                        nc.sync.dma_start(
                            out=sc_t[:rows, :], in_=sc_all[r0:r0 + rows, :])
                        ups = p1ps.tile([128, NPS], f32, tag="ups")
                        nc.tensor.matmul(ups[:rows, 0:NCOL], lhsT=xt_m[:, :rows],
                                         rhs=wnode_m[:], start=True, stop=True)
                        nc.tensor.matmul(ups[:rows, NCOL:NPS], lhsT=pt_m[:, :rows],
                                         rhs=wp1_m[:], start=True, stop=True)
                        uvp_s = p1.tile([128, NPS], f32, tag="uvp")
                        nc.vector.tensor_scalar(uvp_s[:rows, 0:NCOL],
                                                ups[:rows, 0:NCOL],
                                                sc_t[:rows, 0:1], None,
                                                op0=ALU.mult)
                        nc.vector.tensor_copy(uvp_s[:rows, NCOL:NPS],
                                              ups[:rows, NCOL:NPS])
                        # G = [U | P1], S = [V | H | P1] (psum cols 64:320)
                        nc.sync.dma_start(out=G_d[r0:r0 + rows, 0:AH],
                                          in_=uvp_s[:rows, 0:AH])
                        nc.sync.dma_start(out=G_d[r0:r0 + rows, AH:AH + PH],
                                          in_=uvp_s[:rows, NCOL:NPS])
                        nc.sync.dma_start(out=S_d[r0:r0 + rows, :],
                                          in_=uvp_s[:rows, AH:NPS])

            # ------------- phase 2: edges -------------
            with tc.tile_pool(name="eb", bufs=3) as eb, \
                 tc.tile_pool(name="ebg", bufs=3) as ebg, \
                 tc.tile_pool(name="ps_acc", bufs=1, space="PSUM") as ps_acc, \
                 tc.tile_pool(name="ps_du", bufs=1, space="PSUM") as ps_du, \
                 tc.tile_pool(name="ps_dp", bufs=1, space="PSUM") as ps_dp, \
                 tc.tile_pool(name="ps_b", bufs=1, space="PSUM") as ps_b, \
                 tc.tile_pool(name="ps_c", bufs=1, space="PSUM") as ps_c, \
                 tc.tile_pool(name="ps_n", bufs=1, space="PSUM") as ps_n, \
                 tc.tile_pool(name="ps_t", bufs=2, space="PSUM") as ps_t:
                for k in range(nchunk):
                    src16 = eb.tile([128, cfg.T], u16, tag="src16")
                    nc.sync.dma_start(out=src16[:], in_=src_d[k, :, :])
                    src_s = eb.tile([128, cfg.T], i32, tag="src")
                    nc.vector.tensor_copy(src_s[:], src16[:])
                    or16 = eb.tile([128, 1], u16, tag="or16")
                    nc.sync.dma_start(out=or16[:], in_=or_d[k, :, None])
                    or_s = eb.tile([128, 1], i32, tag="or")
                    nc.vector.tensor_copy(or_s[:], or16[:])
                    org_s = eb.tile([128, 1], i32, tag="org")
                    nc.vector.tensor_tensor(org_s[:], or_s[:], base_s[:],
                                            op=ALU.add)
                    cum16 = eb.tile([128, 2], u16, tag="cum16")
                    nc.sync.dma_start(out=cum16[:], in_=cum_d[k, :, :])
                    cum_f = eb.tile([128, 2], f32, tag="cum")
                    nc.vector.tensor_copy(cum_f[:], cum16[:])
                    gl_s = eb.tile([128, AH + PH], f32, tag="gl")
                    nc.gpsimd.indirect_dma_start(
                        out=gl_s[:], out_offset=None, in_=G_d[:],
                        in_offset=IndirectOffsetOnAxis(ap=org_s[:, 0:1], axis=0))
                    gl_m = eb.tile([128, AH + PH], mdt, tag="glm")
                    nc.vector.tensor_copy(gl_m[:], gl_s[:])

                    acc_p = ps_acc.tile([128, 2 * C], f32, tag="acc")

                    for b in range(NB):
                        esl = slice(b * BLK, (b + 1) * BLK)
                        # per-edge src gathers for this block
                        svhs = []
                        for tt in range(cfg.TB):
                            ti = b * cfg.TB + tt
                            svh_t = ebg.tile([128, AH + C + PH], f32, tag=f"svh{tt}")
                            nc.gpsimd.indirect_dma_start(
                                out=svh_t[:], out_offset=None, in_=S_d[:],
                                in_offset=IndirectOffsetOnAxis(
                                    ap=src_s[:, ti:ti + 1], axis=0))
                            svhs.append(svh_t)

                        # indicator indT[n, e] from cumulative degrees
                        ge_a = eb.tile([128, BLK], mdt, tag="gea")
                        nc.vector.tensor_scalar(ge_a[:], iot_f[:, esl],
                                                cum_f[:, 0:1], None, op0=ALU.is_ge)
                        ge_b = eb.tile([128, BLK], mdt, tag="geb")
                        nc.vector.tensor_scalar(ge_b[:], iot_f[:, esl],
                                                cum_f[:, 1:2], None, op0=ALU.is_ge)
                        indT_s = eb.tile([128, BLK], mdt, tag="indT")
                        nc.vector.tensor_tensor(indT_s[:], ge_a[:], ge_b[:],
                                                op=ALU.subtract)

                        # dst-side distribute: Du = U^T ind, Dp = P1^T ind
                        du_p = ps_du.tile([AH, BLK], f32, tag="du")
                        nc.tensor.matmul(du_p[:], lhsT=gl_m[:, 0:AH],
                                         rhs=indT_s[:], start=True, stop=True)
                        dp_p = ps_dp.tile([PH, BLK], f32, tag="dp")
                        nc.tensor.matmul(dp_p[:], lhsT=gl_m[:, AH:AH + PH],
                                         rhs=indT_s[:], start=True, stop=True)

                        # src-side transposes; gd = Du - V^T, pd1 = Dp - P1s^T
                        gd_s = eb.tile([AH, BLK], f32, tag="gd")
                        pd_s = eb.tile([PH, BLK], f32, tag="pd")
                        for tt in range(cfg.TB):
                            csl = slice(tt * 128, (tt + 1) * 128)
                            vT_p = ps_t.tile([128, 128], f32, tag="tr")
                            nc.tensor.transpose(vT_p[:AH, :], svhs[tt][:, 0:AH],
                                                ident_s[:])
                            vT_s = eb.tile([AH, 128], f32, tag="vT")
                            nc.scalar.activation(vT_s[:], vT_p[:AH, :], AF.Copy)
                            nc.vector.tensor_tensor(gd_s[:, csl], du_p[:, csl],
                                                    vT_s[:], op=ALU.subtract)
                            pT_p = ps_t.tile([128, 128], f32, tag="tr")
                            nc.tensor.transpose(
                                pT_p[:PH, :], svhs[tt][:, AH + C:AH + C + PH],
                                ident_s[:])
                            pT_s = eb.tile([PH, 128], f32, tag="pT")
                            nc.scalar.activation(pT_s[:], pT_p[:PH, :], AF.Copy)
                            nc.vector.tensor_tensor(pd_s[:, csl], dp_p[:, csl],
                                                    pT_s[:], op=ALU.subtract)

                        # pos MLP layer 2 (layer 1 was folded through tables)
                        tp1_s = eb.tile([PH, BLK], mdt, tag="tp1")
                        nc.scalar.activation(tp1_s[:], pd_s[:], AF.Relu,
                                             bias=bias_s[0:PH, 0:1])
                        del_p = ps_b.tile([C, BLK], f32, tag="delp")
                        nc.tensor.matmul(del_p[:], lhsT=wp2_m[:],
                                         rhs=tp1_s[:], start=True, stop=True)
                        del_s = eb.tile([C, BLK], f32, tag="dels")
                        nc.scalar.activation(del_s[:], del_p[:], AF.Relu,
                                             bias=bias_s[:, 1:2])
                        del_m = eb.tile([C, BLK], mdt, tag="delm")
                        nc.scalar.activation(del_m[:], del_p[:], AF.Relu,
                                             bias=bias_s[:, 1:2])

                        # attn layer 1: t_a = relu(Wa1^T delta + gd + ba1)
                        z1_p = ps_n.tile([AH, BLK], f32, tag="z1")
                        nc.tensor.matmul(z1_p[:], lhsT=wa1_m[:],
                                         rhs=del_m[:], start=True, stop=True)
                        tsum_s = eb.tile([AH, BLK], f32, tag="tsum")
                        nc.vector.tensor_tensor(tsum_s[:], z1_p[:], gd_s[:],
                                                op=ALU.add)
                        ta_s = eb.tile([AH, BLK], mdt, tag="ta")
                        nc.scalar.activation(ta_s[:], tsum_s[:], AF.Relu,
                                             bias=bias_s[0:AH, 2:3])

                        # attn layer 2 + exp
                        al_p = ps_c.tile([C, BLK], f32, tag="al")
                        nc.tensor.matmul(al_p[:], lhsT=wa2_m[:],
                                         rhs=ta_s[:], start=True, stop=True)
                        ar_s = eb.tile([C, BLK], f32, tag="ar")
                        nc.scalar.activation(ar_s[:], al_p[:], AF.Relu,
                                             bias=bias_s[:, 3:4])
                        e_s = eb.tile([C, BLK], f32, tag="e")
                        nc.scalar.activation(e_s[:], ar_s[:], AF.Exp,
                                             bias=bias_s[:, 4:5])
                        ew2_s = eb.tile([C, BLK], f32, tag="ew2")
                        nc.vector.tensor_tensor(ew2_s[:], e_s[:], del_s[:],
                                                op=ALU.mult)

                        # per-tile: transpose, assemble [ew | e]^T, seg-matmul
                        for tt in range(cfg.TB):
                            ti = b * cfg.TB + tt
                            csl = slice(tt * 128, (tt + 1) * 128)
                            eT_p = ps_t.tile([128, 128], f32, tag="tr")
                            nc.tensor.transpose(eT_p[:], e_s[:, csl], ident_s[:])
                            ew2T_p = ps_t.tile([128, 128], f32, tag="tr")
                            nc.tensor.transpose(ew2T_p[:], ew2_s[:, csl], ident_s[:])
                            iT_p = ps_t.tile([128, 128], mdt, tag="tr")
                            nc.tensor.transpose(iT_p[:], indT_s[:, csl], ident_r[:])
                            ind_s = eb.tile([128, 128], mdt, tag="ind")
                            nc.scalar.activation(ind_s[:], iT_p[:], AF.Copy)
                            ewe_s = eb.tile([128, 2 * C], mdt, tag="ewe")
                            nc.vector.tensor_copy(ewe_s[:, C:], eT_p[:])
                            tmp_s = eb.tile([128, C], f32, tag="tmp")
                            nc.vector.tensor_tensor(tmp_s[:], eT_p[:],
                                                    svhs[tt][:, AH:AH + C],
                                                    op=ALU.mult)
                            nc.vector.tensor_tensor(ewe_s[:, 0:C], tmp_s[:],
                                                    ew2T_p[:], op=ALU.add)
                            nc.tensor.matmul(acc_p[:], lhsT=ind_s[:],
                                             rhs=ewe_s[:],
                                             start=(ti == 0), stop=(ti == cfg.T - 1))

                    # finalize chunk
                    sp_s = eb.tile([128, C], f32, tag="sp")
                    nc.vector.tensor_scalar_add(sp_s[:], acc_p[:, C:], cfg.EPS)
                    rp_s = eb.tile([128, C], f32, tag="rp")
                    nc.vector.reciprocal(rp_s[:], sp_s[:])
                    o_s = eb.tile([128, C], f32, tag="o")
                    nc.vector.tensor_tensor(o_s[:], acc_p[:, 0:C], rp_s[:],
                                            op=ALU.mult)
                    o2_s = eb.tile([128, C], y_dt, tag="o2")
                    if cfg.y_mode == "u8":
                        # f32->u8 conversion rounds to nearest
                        nc.scalar.activation(o2_s[:], o_s[:], AF.Relu,
                                             scale=1.0 / cfg.Y_SCALE)
                    else:
                        nc.scalar.activation(o2_s[:], o_s[:], AF.Relu)
                    nc.gpsimd.indirect_dma_start(
                        out=y_d[:], out_offset=IndirectOffsetOnAxis(ap=or_s[:, :1], axis=0),
                        in_=o2_s[:], in_offset=None)
    nc.finalize()
    return nc


def _build_inputs(inputs, cfg):
    x = np.ascontiguousarray(np.asarray(inputs["x"], np.float32))
    pos = np.ascontiguousarray(np.asarray(inputs["pos"], np.float32))
    W_lin = np.asarray(inputs["W_lin"], np.float32)
    W_src = np.asarray(inputs["W_src"], np.float32)
    W_dst = np.asarray(inputs["W_dst"], np.float32)
    Wp1 = np.asarray(inputs["Wp1"], np.float32)
    bp1 = np.asarray(inputs["bp1"], np.float32)
    Wp2 = np.asarray(inputs["Wp2"], np.float32)
    bp2 = np.asarray(inputs["bp2"], np.float32)
    Wa1 = np.asarray(inputs["Wa1"], np.float32)
    ba1 = np.asarray(inputs["ba1"], np.float32)
    Wa2 = np.asarray(inputs["Wa2"], np.float32)
    ba2 = np.asarray(inputs["ba2"], np.float32)

    Wda = (W_dst @ Wa1).astype(np.float32)   # [C, AH]
    Wsa = (W_src @ Wa1).astype(np.float32)
    wnode = np.concatenate([Wda, Wsa, W_lin], axis=1)  # [C, 2AH + C]
    bias = np.zeros((128, 5), np.float32)
    bias[: cfg.PH, 0] = bp1
    bias[: cfg.C, 1] = bp2
    bias[: cfg.AH, 2] = ba1
    bias[: cfg.C, 3] = ba2
    bias[:, 4] = -cfg.SHIFT

    # x int8 quantization with per-node scale
    xsc = np.maximum(np.abs(x).max(axis=1, keepdims=True), 1e-12) / 127.0
    xq = np.clip(np.round(x / xsc), -127, 127).astype(np.int8)
    # pos u16 codes; dequant folded into Wp1
    pq = np.clip(np.round(pos * 65535.0), 0, 65535).astype(np.uint16)

    packs, nchunk = _pack(x, pos, inputs["edge_index"], cfg)
    enc = _to_bf16_bits if cfg.w_bf16 else np.ascontiguousarray
    common = dict(Wnode=enc(wnode),
                  Wp1=np.ascontiguousarray(Wp1 / np.float32(65535.0)),
                  Wp2=enc(Wp2), Wa2=enc(Wa2), Wa1p=enc(Wa1), bias=bias)
    in_maps = []
    for c, p in enumerate(packs):
        sl = slice(c * cfg.NSH, (c + 1) * cfg.NSH)
        m = dict(common,
                 xqsh=np.ascontiguousarray(xq[sl].T),
                 xsc=np.ascontiguousarray(xsc[sl].astype(np.float32)),
                 posT=np.ascontiguousarray(pq[sl].T),
                 basec=np.full((128, 1), c * cfg.NLOC, np.int32),
                 **p)
        in_maps.append(m)
    return in_maps, nchunk


def decode_y(y):
    if CFG.y_mode == "u8":
        return y.astype(np.float32) * np.float32(CFG.Y_SCALE)
    return y


# ---------------------------------------------------------------- runner
# Mirror of bass2jax.run_bass_via_pjrt, with two wall-clock fixes for the
# per-call path:
#   * the jitted shard_map executable is built ONCE and cached (the stock
#     helper re-jits a fresh closure every call -> ~1.3s of retrace/XLA
#     re-lowering per call),
#   * the zero-initialized ExternalOutput buffers are device-resident and
#     reused (not re-uploaded per call; the custom call copies them into
#     the result buffer device-side).
# Every call still ships all in_map bytes host->device, executes, and
# fetches the outputs back to numpy.
_RUNNER = {}


def _make_runner(nc, n_cores):
    import jax
    from jax.sharding import Mesh, PartitionSpec, NamedSharding
    from jax.experimental.shard_map import shard_map
    from concourse.bass2jax import (
        _bass_exec_p, partition_id_tensor, install_neuronx_cc_hook)

    install_neuronx_cc_hook()
    assert not nc.dbg_callbacks
    partition_name = (
        nc.partition_id_tensor.name if nc.partition_id_tensor else None)
    in_names, out_names, out_avals, zero_outs = [], [], [], []
    for alloc in nc.m.functions[0].allocations:
        if not isinstance(alloc, mybir.MemoryLocationSet):
            continue
        name = alloc.memorylocations[0].name
        if alloc.kind == "ExternalInput":
            if name != partition_name and name != (
                    nc.dbg_addr.name if nc.dbg_addr is not None else None):
                in_names.append(name)
        elif alloc.kind == "ExternalOutput":
            shape = tuple(alloc.tensor_shape)
            dtype = mybir.dt.np(alloc.dtype)
            out_avals.append(jax.core.ShapedArray(shape, dtype))
            zero_outs.append(np.zeros(shape, dtype))
            out_names.append(name)
    n_params = len(in_names)
    in_names_all = list(in_names) + out_names
    if nc.dbg_addr is not None:
        in_names_all.append(nc.dbg_addr.name)
    if partition_name is not None:
        in_names_all.append(partition_name)

    def _body(*args):
        operands = list(args)
        if nc.dbg_addr is not None:
            operands.append(jax.numpy.zeros((1, 2), jax.numpy.uint32))
        if partition_name is not None:
            operands.append(partition_id_tensor())
        return tuple(_bass_exec_p.bind(
            *operands, out_avals=tuple(out_avals),
            in_names=tuple(in_names_all), out_names=tuple(out_names),
            lowering_input_output_aliases=(),
            sim_require_finite=True, sim_require_nnan=True, nc=nc))

    devices = jax.devices()[:n_cores]
    mesh = Mesh(np.asarray(devices), ("core",))
    nsh = NamedSharding(mesh, PartitionSpec("core"))
    n_outs = len(out_avals)
    sharded = jax.jit(
        shard_map(_body, mesh=mesh,
                  in_specs=(PartitionSpec("core"),) * (n_params + n_outs),
                  out_specs=(PartitionSpec("core"),) * n_outs,
                  check_rep=False),
        keep_unused=True)
    dev_zeros = [
        jax.device_put(
            np.zeros((n_cores * z.shape[0], *z.shape[1:]), z.dtype), nsh)
        for z in zero_outs]

    def run(in_maps):
        concat_in = [
            np.concatenate([np.asarray(m[nm]) for m in in_maps], axis=0)
            for nm in in_names]
        out_arrs = sharded(*concat_in, *dev_zeros)
        return [
            {name: np.asarray(out_arrs[i]).reshape(
                n_cores, *out_avals[i].shape)[c]
             for i, name in enumerate(out_names)}
            for c in range(n_cores)]

    return run


def run_cached(nc, in_maps, n_cores):
    key = id(nc)
    if key not in _RUNNER:
        _RUNNER[key] = _make_runner(nc, n_cores)
    return _RUNNER[key](in_maps)


def kernel(**inputs):
    cfg = CFG
    in_maps, nchunk = _build_inputs(inputs, cfg)
    nc = _build(cfg, nchunk)
    results = run_cached(nc, in_maps, cfg.M)
    y = np.concatenate(
        [results[c]["y"][: cfg.NLOC] for c in range(cfg.M)], axis=0)
    return decode_y(y)


# revision 18
# speedup vs baseline: 6.0180x; 1.2083x over previous
"""Trainium2 Bass kernel for nn_ClusterEncoder (PointTransformerConv-style
GNN message passing), 8-core SPMD.

The metric regime is axon-tunnel transfer-bound (tens of MB/s host<->device
plus ~80ms fixed dispatch), so the kernel minimizes per-call host bytes and
per-call Python/XLA overhead:
  * x ships as int8 with a per-node f32 scale (xT shard [128, 6250] i8 +
    scale [6250,1] f32 per core); full tables are assembled on device with
    AllGather collectives and the scale is applied post-matmul (U/V/H rows
    scale linearly in x_n).
  * pos ships as u16 codes (pos*65535), sharded + AllGathered; the 1/65535
    dequant is folded into Wp1 host-side.
  * No per-edge dst metadata: edges are sorted by dst and grouped into
    <=128-node chunks; the per-chunk edge->node indicator matrix is built
    on device from cumulative degrees (u16 [128,2] per chunk) and an iota,
    replacing the dstid/dstloc uploads. posd is never shipped: the pos MLP
    layer-1 (pos @ Wp1) is computed per node in phase 1 and distributed to
    edges through the same indicator matmul (it is linear, so
    P1[dst]-P1[src] can replace Wp1^T posd).
  * y returns as u8 (scale 2.75/255), decoded on host.
  * run_cached keeps ONE jitted shard_map executable alive across calls and
    keeps the zero-filled output donation buffers device-resident, so a
    steady-state call is: concat numpy inputs -> h2d -> exec -> d2h.

Compute strategy (edges sharded by destination node):
  * Host: sort edges by dst, split nodes into 8 equal contiguous ranges.
    Greedy-pack destination nodes into chunks of <=128 nodes / <=CHUNK_E
    edges; ship per-chunk src ids (u16), local out rows (u16), and
    cumulative-degree pairs (u16).
  * Device, phase 0: AllGather x int8 shards, x scales, posT u16 shards.
  * Device, phase 1 (replicated): per 128-node tile
      uvhp = [x@Wda | x@Wsa | x@Wlin | pos@Wp1'] (two matmuls into one
      PSUM tile), scale the x part per node, write tables
      G = [U | P1]  [N+1, 128] and S = [V | H | P1]  [N, 256] to DRAM.
  * Device, phase 2 (per chunk of 16 x 128-edge tiles):
      - gather G rows once per chunk by outrow (dst nodes), gather S rows
        per edge by src id,
      - indicator indT[n,e] = (e>=cum[n]) - (e>=cum[n+1]) from iota,
      - dst-side distribute: Du = G_U^T @ indT, Dp = G_P1^T @ indT,
      - t_p1 = relu(Dp - P1s^T + bp1);  delta = relu(Wp2^T t_p1 + bp2),
      - t_a = relu(Wa1^T delta + (Du - V^T) + ba1),
      - logits = relu(Wa2^T t_a + ba2);  e = exp(logits - SHIFT)
        (constant shift cancels in e/sum(e); logits are relu-bounded),
      - seg-sum via matmul: acc[n,0:128] += ind^T @ (e*(H[src]+delta))^T,
        acc[n,128:256] += ind^T @ e^T,
      - out = relu(NUM / (s + eps)); indirect-scatter rows to y (u8).
  * Softmax segments are core-local by construction (edges sharded by dst).
"""
import sys
from dataclasses import dataclass
from math import ceil

if "/opt/trn_rl_repo" not in sys.path:
    sys.path.insert(0, "/opt/trn_rl_repo")

import ml_dtypes
import numpy as np

import concourse.bass as bass
import concourse.mybir as mybir
import concourse.tile as tile
from concourse import bacc
from concourse.bass import IndirectOffsetOnAxis
from concourse.bass_utils import run_bass_kernel_spmd
from concourse.masks import make_identity

f32 = mybir.dt.float32
f32r = mybir.dt.float32r
bf16 = mybir.dt.bfloat16
i32 = mybir.dt.int32
u16 = mybir.dt.uint16
i8 = mybir.dt.int8
AF = mybir.ActivationFunctionType
ALU = mybir.AluOpType
BF16 = ml_dtypes.bfloat16


@dataclass
class Cfg:
    N: int = 50000
    C: int = 128
    PH: int = 64
    AH: int = 64
    DIM: int = 2
    M: int = 8            # cores
    T: int = 16           # 128-edge tiles per chunk
    TB: int = 4           # tiles per matmul block (block = 512 edges)
    SHIFT: float = 8.0
    EPS: float = 1e-12
    mm_dt: object = f32r  # matmul compute dtype (f32r: 1 cyc/row at free>=256)
    y_mode: str = "u6"    # "u8" | "u6" output encoding
    Y_SCALE: float = 2.75 / 255.0  # u8 quant step (y in [0, ~2.44])
    KWSH: int = 65        # weight-blob columns per core shard (8*65 = 520)

    @property
    def NLOC(self):
        return self.N // self.M

    @property
    def NSH(self):
        return self.N // self.M  # x rows per core shard

    @property
    def CHUNK_E(self):
        return self.T * 128

    @property
    def OUT_ROWS(self):
        return self.NLOC + 1  # +1 trash row for padded scatter lanes


CFG = Cfg()


def _to_bf16_bits(a):
    return np.ascontiguousarray(a.astype(BF16).view(np.uint16))


# ---------------------------------------------------------------- host pack
def _pack(x, pos, edge_index, cfg):
    """Sort/shard/chunk edges; returns per-core input dicts (minus weights)."""
    src = np.asarray(edge_index[0], np.int64)
    dst = np.asarray(edge_index[1], np.int64)
    order = np.argsort(dst, kind="stable")
    s_s = src[order]
    d_s = dst[order]

    NLOC = cfg.NLOC
    bounds = np.searchsorted(d_s, np.arange(cfg.M + 1) * NLOC)

    cores = []
    for c in range(cfg.M):
        lo, hi = bounds[c], bounds[c + 1]
        dloc = d_s[lo:hi] - c * NLOC
        deg = np.bincount(dloc, minlength=NLOC)
        nodes = np.nonzero(deg)[0]
        chunks = []  # (node_list, e0, e1) ; e relative to lo
        cur, cur_e, estart = [], 0, 0
        for n in nodes:
            dn = int(deg[n])
            assert dn <= cfg.CHUNK_E, f"degree {dn} exceeds chunk capacity"
            if len(cur) == 128 or cur_e + dn > cfg.CHUNK_E:
                chunks.append((cur, estart, estart + cur_e))
                estart += cur_e
                cur, cur_e = [], 0
            cur.append(int(n))
            cur_e += dn
        if cur:
            chunks.append((cur, estart, estart + cur_e))
        cores.append((lo, chunks, deg))

    NCHUNK = max(max(len(ch) for _, ch, _ in cores), 1)

    in_maps = []
    for c in range(cfg.M):
        lo, chunks, deg = cores[c]
        srcid = np.zeros((NCHUNK, 128, cfg.T), np.uint16)
        cum129 = np.zeros((NCHUNK, 129, 1), np.uint16)
        outrow = np.full((NCHUNK, 128), cfg.NLOC, np.uint16)  # trash row
        for k, (nl, e0, e1) in enumerate(chunks):
            cnt = e1 - e0
            g0, g1 = lo + e0, lo + e1
            nla = np.asarray(nl, np.int64)
            j = np.arange(cnt)
            srcid[k, j & 127, j >> 7] = s_s[g0:g1].astype(np.uint16)
            cums = np.concatenate([[0], np.cumsum(deg[nla])])
            cum129[k, :, 0] = cnt
            cum129[k, : len(nl) + 1, 0] = cums
            outrow[k, : len(nl)] = nla.astype(np.uint16)
        in_maps.append(dict(srcid=srcid, cum129=cum129, outrow=outrow))
    return in_maps, NCHUNK


# ---------------------------------------------------------------- program
def _build(cfg, nchunk):
    nc = bacc.Bacc(None, target_bir_lowering=False, num_devices=cfg.M)
    N, C, PH, AH, DIM = cfg.N, cfg.C, cfg.PH, cfg.AH, cfg.DIM
    NSH = cfg.NSH
    mdt = cfg.mm_dt

    u8 = mybir.dt.uint8
    KW = cfg.M * cfg.KWSH  # 520 weight-blob columns
    xq_d = nc.declare_dram_parameter("xqsh", [C, NSH], i8, isOutput=False)
    xsc_d = nc.declare_dram_parameter("xsc", [NSH, 1], f32, isOutput=False)
    pos_d = nc.declare_dram_parameter("posT", [DIM, NSH], u16, isOutput=False)
    wb_d = nc.declare_dram_parameter("wblob", [128, cfg.KWSH], bf16, isOutput=False)
    src_d = nc.declare_dram_parameter("srcid", [nchunk, 128, cfg.T], u16, isOutput=False)
    cum_d = nc.declare_dram_parameter("cum129", [nchunk, 129, 1], u16, isOutput=False)
    or_d = nc.declare_dram_parameter("outrow", [nchunk, 128], u16, isOutput=False)
    base_d = nc.declare_dram_parameter("basec", [128, 1], i32, isOutput=False)
    if cfg.y_mode == "u6":
        y_d = nc.declare_dram_parameter("y", [cfg.OUT_ROWS, 96], u8, isOutput=True)
        ysc_d = nc.declare_dram_parameter("ysc", [cfg.OUT_ROWS, 1], f32, isOutput=True)
    else:
        y_d = nc.declare_dram_parameter("y", [cfg.OUT_ROWS, C], u8, isOutput=True)

    # node tables: G = [U | P1] (dst side), S = [V | H | P1] (src side)
    G_d = nc.dram_tensor("G", [N + 1, AH + PH], f32)
    S_d = nc.dram_tensor("S", [N, AH + C + PH], f32)
    xg_in = nc.dram_tensor("xg_in", [C, NSH], i8)
    xq_all = nc.dram_tensor("xq_all", [cfg.M * C, NSH], i8)
    sc_in = nc.dram_tensor("sc_in", [NSH, 1], f32)
    sc_all = nc.dram_tensor("sc_all", [cfg.M * NSH, 1], f32)
    pg_in = nc.dram_tensor("pg_in", [DIM, NSH], u16)
    pos_all = nc.dram_tensor("pos_all", [cfg.M * DIM, NSH], u16)
    wb_in = nc.dram_tensor("wb_in", [128, cfg.KWSH], bf16)
    wb_all = nc.dram_tensor("wb_all", [cfg.M * 128, cfg.KWSH], bf16)

    NB = cfg.T // cfg.TB  # blocks per chunk
    BLK = cfg.TB * 128
    NCOL = 2 * AH + C  # 256: U | V | H psum columns
    NPS = NCOL + PH    # 320: + P1

    with tile.TileContext(nc) as tc:
        with tc.tile_pool(name="const", bufs=1) as cp:
            base_s = cp.tile([128, 1], i32)
            nc.sync.dma_start(out=base_s[:], in_=base_d[:, :])
            ident_s = cp.tile([128, 128], f32)
            make_identity(nc, ident_s[:])
            ident_r = cp.tile([128, 128], mdt)
            nc.vector.tensor_copy(ident_r[:], ident_s[:])
            iot_i = cp.tile([128, cfg.CHUNK_E], i32)
            nc.gpsimd.iota(iot_i[:], pattern=[[1, cfg.CHUNK_E]], base=0,
                           channel_multiplier=0)
            iot_f = cp.tile([128, cfg.CHUNK_E], f32)
            nc.vector.tensor_copy(iot_f[:], iot_i[:])

            # ------------- phase 0: AllGather weights / x / scales / pos ---
            groups = [list(range(cfg.M))]
            nc.gpsimd.dma_start(out=wb_in[:, :], in_=wb_d[:, :])
            nc.gpsimd.collective_compute(
                "AllGather", mybir.AluOpType.bypass, replica_groups=groups,
                ins=[wb_in[:, :]], outs=[wb_all[:, :]])
            nc.gpsimd.dma_start(out=xg_in[:, :], in_=xq_d[:, :])
            nc.gpsimd.collective_compute(
                "AllGather", mybir.AluOpType.bypass, replica_groups=groups,
                ins=[xg_in[:, :]], outs=[xq_all[:, :]])
            nc.gpsimd.dma_start(out=sc_in[:, :], in_=xsc_d[:, :])
            nc.gpsimd.collective_compute(
                "AllGather", mybir.AluOpType.bypass, replica_groups=groups,
                ins=[sc_in[:, :]], outs=[sc_all[:, :]])
            nc.gpsimd.dma_start(out=pg_in[:, :], in_=pos_d[:, :])
            nc.gpsimd.collective_compute(
                "AllGather", mybir.AluOpType.bypass, replica_groups=groups,
                ins=[pg_in[:, :]], outs=[pos_all[:, :]])

            # reassemble the weight blob [128, KW] from the gathered shards
            # blob cols: 0:256 wnode | 256:320 wa1 | 320:384 wp2^T
            #            | 384:448 wa2^T | 448:453 bias | 456:520 wp1 (rows 0:2)
            wb_s = cp.tile([128, KW], bf16)
            for c8 in range(cfg.M):
                nc.sync.dma_start(
                    out=wb_s[:, c8 * cfg.KWSH:(c8 + 1) * cfg.KWSH],
                    in_=wb_all[c8 * 128:(c8 + 1) * 128, :])
            wnode_m = cp.tile([C, NCOL], mdt)
            nc.vector.tensor_copy(wnode_m[:], wb_s[:, 0:256])
            wa1_m = cp.tile([C, AH], mdt)
            nc.vector.tensor_copy(wa1_m[:], wb_s[:, 256:320])
            bias_s = cp.tile([128, 5], f32)
            nc.vector.tensor_copy(bias_s[:], wb_s[:, 448:453])
            wp1_m = cp.tile([DIM, PH], mdt)
            nc.vector.tensor_copy(wp1_m[:], wb_s[0:DIM, 456:520])
            wp2t_f = cp.tile([C, PH], f32)
            nc.vector.tensor_copy(wp2t_f[:], wb_s[:, 320:384])
            wa2t_f = cp.tile([C, AH], f32)
            nc.vector.tensor_copy(wa2t_f[:], wb_s[:, 384:448])
            wp2_m = cp.tile([PH, C], mdt)
            wa2_m = cp.tile([AH, C], mdt)
            with tc.tile_pool(name="wtp", bufs=1, space="PSUM") as wtp:
                wt_p = wtp.tile([128, 128], f32, tag="wt")
                nc.tensor.transpose(wt_p[:PH, :], wp2t_f[:], ident_s[:])
                nc.scalar.activation(wp2_m[:], wt_p[:PH, :], AF.Copy)
                wt2_p = wtp.tile([128, 128], f32, tag="wt2")
                nc.tensor.transpose(wt2_p[:AH, :], wa2t_f[:], ident_s[:])
                nc.scalar.activation(wa2_m[:], wt2_p[:AH, :], AF.Copy)

            # ------------- phase 1: node tables G / S -------------
            with tc.tile_pool(name="p1", bufs=3) as p1, \
                 tc.tile_pool(name="p1ps", bufs=2, space="PSUM") as p1ps:
                # zero the trash row N of G (gathered by padded outrow slots)
                zrow = p1.tile([1, AH + PH], f32, tag="zrow")
                nc.vector.memset(zrow[:], 0.0)
                nc.sync.dma_start(out=G_d[N:N + 1, :], in_=zrow[:])
                ncol = ceil(NSH / 128)
                for c8 in range(cfg.M):
                    for t in range(ncol):
                        j0 = t * 128
                        rows = min(128, NSH - j0)
                        r0 = c8 * NSH + j0
                        xt = p1.tile([C, 128], i8, tag="xt")
                        nc.sync.dma_start(
                            out=xt[:, :rows],
                            in_=xq_all[c8 * C:(c8 + 1) * C, j0:j0 + rows])
                        xt_m = p1.tile([C, 128], mdt, tag="xtm")
                        nc.vector.tensor_copy(xt_m[:, :rows], xt[:, :rows])
                        pt = p1.tile([DIM, 128], u16, tag="pt")
                        nc.sync.dma_start(
                            out=pt[:, :rows],
                            in_=pos_all[c8 * DIM:(c8 + 1) * DIM, j0:j0 + rows])
                        pt_m = p1.tile([DIM, 128], mdt, tag="ptm")
                        nc.vector.tensor_copy(pt_m[:, :rows], pt[:, :rows])
                        sc_t = p1.tile([128, 1], f32, tag="sc")
                        nc.sync.dma_start(
                            out=sc_t[:rows, :], in_=sc_all[r0:r0 + rows, :])
                        ups = p1ps.tile([128, NPS], f32, tag="ups")
                        nc.tensor.matmul(ups[:rows, 0:NCOL], lhsT=xt_m[:, :rows],
                                         rhs=wnode_m[:], start=True, stop=True)
                        nc.tensor.matmul(ups[:rows, NCOL:NPS], lhsT=pt_m[:, :rows],
                                         rhs=wp1_m[:], start=True, stop=True)
                        uvp_s = p1.tile([128, NPS], f32, tag="uvp")
                        nc.vector.tensor_scalar(uvp_s[:rows, 0:NCOL],
                                                ups[:rows, 0:NCOL],
                                                sc_t[:rows, 0:1], None,
                                                op0=ALU.mult)
                        nc.vector.tensor_copy(uvp_s[:rows, NCOL:NPS],
                                              ups[:rows, NCOL:NPS])
                        # G = [U | P1], S = [V | H | P1] (psum cols 64:320)
                        nc.sync.dma_start(out=G_d[r0:r0 + rows, 0:AH],
                                          in_=uvp_s[:rows, 0:AH])
                        nc.sync.dma_start(out=G_d[r0:r0 + rows, AH:AH + PH],
                                          in_=uvp_s[:rows, NCOL:NPS])
                        nc.sync.dma_start(out=S_d[r0:r0 + rows, :],
                                          in_=uvp_s[:rows, AH:NPS])

            # ------------- phase 2: edges -------------
            with tc.tile_pool(name="eb", bufs=3) as eb, \
                 tc.tile_pool(name="ebg", bufs=3) as ebg, \
                 tc.tile_pool(name="ps_acc", bufs=1, space="PSUM") as ps_acc, \
                 tc.tile_pool(name="ps_du", bufs=1, space="PSUM") as ps_du, \
                 tc.tile_pool(name="ps_dp", bufs=1, space="PSUM") as ps_dp, \
                 tc.tile_pool(name="ps_b", bufs=1, space="PSUM") as ps_b, \
                 tc.tile_pool(name="ps_c", bufs=1, space="PSUM") as ps_c, \
                 tc.tile_pool(name="ps_n", bufs=1, space="PSUM") as ps_n, \
                 tc.tile_pool(name="ps_t", bufs=2, space="PSUM") as ps_t:
                for k in range(nchunk):
                    src16 = eb.tile([128, cfg.T], u16, tag="src16")
                    nc.sync.dma_start(out=src16[:], in_=src_d[k, :, :])
                    src_s = eb.tile([128, cfg.T], i32, tag="src")
                    nc.vector.tensor_copy(src_s[:], src16[:])
                    or16 = eb.tile([128, 1], u16, tag="or16")
                    nc.sync.dma_start(out=or16[:], in_=or_d[k, :, None])
                    or_s = eb.tile([128, 1], i32, tag="or")
                    nc.vector.tensor_copy(or_s[:], or16[:])
                    org_s = eb.tile([128, 1], i32, tag="org")
                    nc.vector.tensor_tensor(org_s[:], or_s[:], base_s[:],
                                            op=ALU.add)
                    cum16 = eb.tile([128, 2], u16, tag="cum16")
                    nc.sync.dma_start(out=cum16[:, 0:1], in_=cum_d[k, 0:128, :])
                    nc.sync.dma_start(out=cum16[:, 1:2], in_=cum_d[k, 1:129, :])
                    cum_f = eb.tile([128, 2], f32, tag="cum")
                    nc.vector.tensor_copy(cum_f[:], cum16[:])
                    gl_s = eb.tile([128, AH + PH], f32, tag="gl")
                    nc.gpsimd.indirect_dma_start(
                        out=gl_s[:], out_offset=None, in_=G_d[:],
                        in_offset=IndirectOffsetOnAxis(ap=org_s[:, 0:1], axis=0))
                    gl_m = eb.tile([128, AH + PH], mdt, tag="glm")
                    nc.vector.tensor_copy(gl_m[:], gl_s[:])

                    acc_p = ps_acc.tile([128, 2 * C], f32, tag="acc")

                    for b in range(NB):
                        esl = slice(b * BLK, (b + 1) * BLK)
                        # per-edge src gathers for this block
                        svhs = []
                        for tt in range(cfg.TB):
                            ti = b * cfg.TB + tt
                            svh_t = ebg.tile([128, AH + C + PH], f32, tag=f"svh{tt}")
                            nc.gpsimd.indirect_dma_start(
                                out=svh_t[:], out_offset=None, in_=S_d[:],
                                in_offset=IndirectOffsetOnAxis(
                                    ap=src_s[:, ti:ti + 1], axis=0))
                            svhs.append(svh_t)

                        # indicator indT[n, e] from cumulative degrees
                        ge_a = eb.tile([128, BLK], mdt, tag="gea")
                        nc.vector.tensor_scalar(ge_a[:], iot_f[:, esl],
                                                cum_f[:, 0:1], None, op0=ALU.is_ge)
                        ge_b = eb.tile([128, BLK], mdt, tag="geb")
                        nc.vector.tensor_scalar(ge_b[:], iot_f[:, esl],
                                                cum_f[:, 1:2], None, op0=ALU.is_ge)
                        indT_s = eb.tile([128, BLK], mdt, tag="indT")
                        nc.vector.tensor_tensor(indT_s[:], ge_a[:], ge_b[:],
                                                op=ALU.subtract)

                        # dst-side distribute: Du = U^T ind, Dp = P1^T ind
                        du_p = ps_du.tile([AH, BLK], f32, tag="du")
                        nc.tensor.matmul(du_p[:], lhsT=gl_m[:, 0:AH],
                                         rhs=indT_s[:], start=True, stop=True)
                        dp_p = ps_dp.tile([PH, BLK], f32, tag="dp")
                        nc.tensor.matmul(dp_p[:], lhsT=gl_m[:, AH:AH + PH],
                                         rhs=indT_s[:], start=True, stop=True)

                        # src-side transposes; gd = Du - V^T, pd1 = Dp - P1s^T
                        gd_s = eb.tile([AH, BLK], f32, tag="gd")
                        pd_s = eb.tile([PH, BLK], f32, tag="pd")
                        for tt in range(cfg.TB):
                            csl = slice(tt * 128, (tt + 1) * 128)
                            vT_p = ps_t.tile([128, 128], f32, tag="tr")
                            nc.tensor.transpose(vT_p[:AH, :], svhs[tt][:, 0:AH],
                                                ident_s[:])
                            vT_s = eb.tile([AH, 128], f32, tag="vT")
                            nc.scalar.activation(vT_s[:], vT_p[:AH, :], AF.Copy)
                            nc.vector.tensor_tensor(gd_s[:, csl], du_p[:, csl],
                                                    vT_s[:], op=ALU.subtract)
                            pT_p = ps_t.tile([128, 128], f32, tag="tr")
                            nc.tensor.transpose(
                                pT_p[:PH, :], svhs[tt][:, AH + C:AH + C + PH],
                                ident_s[:])
                            pT_s = eb.tile([PH, 128], f32, tag="pT")
                            nc.scalar.activation(pT_s[:], pT_p[:PH, :], AF.Copy)
                            nc.vector.tensor_tensor(pd_s[:, csl], dp_p[:, csl],
                                                    pT_s[:], op=ALU.subtract)

                        # pos MLP layer 2 (layer 1 was folded through tables)
                        tp1_s = eb.tile([PH, BLK], mdt, tag="tp1")
                        nc.scalar.activation(tp1_s[:], pd_s[:], AF.Relu,
                                             bias=bias_s[0:PH, 0:1])
                        del_p = ps_b.tile([C, BLK], f32, tag="delp")
                        nc.tensor.matmul(del_p[:], lhsT=wp2_m[:],
                                         rhs=tp1_s[:], start=True, stop=True)
                        del_s = eb.tile([C, BLK], f32, tag="dels")
                        nc.scalar.activation(del_s[:], del_p[:], AF.Relu,
                                             bias=bias_s[:, 1:2])
                        del_m = eb.tile([C, BLK], mdt, tag="delm")
                        nc.scalar.activation(del_m[:], del_p[:], AF.Relu,
                                             bias=bias_s[:, 1:2])

                        # attn layer 1: t_a = relu(Wa1^T delta + gd + ba1)
                        z1_p = ps_n.tile([AH, BLK], f32, tag="z1")
                        nc.tensor.matmul(z1_p[:], lhsT=wa1_m[:],
                                         rhs=del_m[:], start=True, stop=True)
                        tsum_s = eb.tile([AH, BLK], f32, tag="tsum")
                        nc.vector.tensor_tensor(tsum_s[:], z1_p[:], gd_s[:],
                                                op=ALU.add)
                        ta_s = eb.tile([AH, BLK], mdt, tag="ta")
                        nc.scalar.activation(ta_s[:], tsum_s[:], AF.Relu,
                                             bias=bias_s[0:AH, 2:3])

                        # attn layer 2 + exp
                        al_p = ps_c.tile([C, BLK], f32, tag="al")
                        nc.tensor.matmul(al_p[:], lhsT=wa2_m[:],
                                         rhs=ta_s[:], start=True, stop=True)
                        ar_s = eb.tile([C, BLK], f32, tag="ar")
                        nc.scalar.activation(ar_s[:], al_p[:], AF.Relu,
                                             bias=bias_s[:, 3:4])
                        e_s = eb.tile([C, BLK], f32, tag="e")
                        nc.scalar.activation(e_s[:], ar_s[:], AF.Exp,
                                             bias=bias_s[:, 4:5])
                        ew2_s = eb.tile([C, BLK], f32, tag="ew2")
                        nc.vector.tensor_tensor(ew2_s[:], e_s[:], del_s[:],
                                                op=ALU.mult)

                        # per-tile: transpose, assemble [ew | e]^T, seg-matmul
                        for tt in range(cfg.TB):
                            ti = b * cfg.TB + tt
                            csl = slice(tt * 128, (tt + 1) * 128)
                            eT_p = ps_t.tile([128, 128], f32, tag="tr")
                            nc.tensor.transpose(eT_p[:], e_s[:, csl], ident_s[:])
                            ew2T_p = ps_t.tile([128, 128], f32, tag="tr")
                            nc.tensor.transpose(ew2T_p[:], ew2_s[:, csl], ident_s[:])
                            iT_p = ps_t.tile([128, 128], mdt, tag="tr")
                            nc.tensor.transpose(iT_p[:], indT_s[:, csl], ident_r[:])
                            ind_s = eb.tile([128, 128], mdt, tag="ind")
                            nc.scalar.activation(ind_s[:], iT_p[:], AF.Copy)
                            ewe_s = eb.tile([128, 2 * C], mdt, tag="ewe")
                            nc.vector.tensor_copy(ewe_s[:, C:], eT_p[:])
                            tmp_s = eb.tile([128, C], f32, tag="tmp")
                            nc.vector.tensor_tensor(tmp_s[:], eT_p[:],
                                                    svhs[tt][:, AH:AH + C],
                                                    op=ALU.mult)
                            nc.vector.tensor_tensor(ewe_s[:, 0:C], tmp_s[:],
                                                    ew2T_p[:], op=ALU.add)
                            nc.tensor.matmul(acc_p[:], lhsT=ind_s[:],
                                             rhs=ewe_s[:],
                                             start=(ti == 0), stop=(ti == cfg.T - 1))

                    # finalize chunk
                    sp_s = eb.tile([128, C], f32, tag="sp")
                    nc.vector.tensor_scalar_add(sp_s[:], acc_p[:, C:], cfg.EPS)
                    rp_s = eb.tile([128, C], f32, tag="rp")
                    nc.vector.reciprocal(rp_s[:], sp_s[:])
                    o_s = eb.tile([128, C], f32, tag="o")
                    nc.vector.tensor_tensor(o_s[:], acc_p[:, 0:C], rp_s[:],
                                            op=ALU.mult)
                    if cfg.y_mode == "u6":
                        # per-row 6-bit encode: code = round(relu(o)*63/rowmax),
                        # pack 4 codes into 3 bytes; ship rowmax/63 as f32.
                        o_r = eb.tile([128, C], f32, tag="orr")
                        nc.scalar.activation(o_r[:], o_s[:], AF.Relu)
                        mx_s = eb.tile([128, 1], f32, tag="mx")
                        nc.vector.reduce_max(mx_s[:], o_r[:],
                                             axis=mybir.AxisListType.X)
                        mp_s = eb.tile([128, 1], f32, tag="mp")
                        nc.vector.tensor_scalar_add(mp_s[:], mx_s[:], 1e-30)
                        rq_s = eb.tile([128, 1], f32, tag="rq")
                        nc.vector.reciprocal(rq_s[:], mp_s[:])
                        r63_s = eb.tile([128, 1], f32, tag="r63")
                        nc.vector.tensor_scalar(r63_s[:], rq_s[:], 63.0, None,
                                                op0=ALU.mult)
                        cf_s = eb.tile([128, C], f32, tag="cf")
                        nc.vector.tensor_scalar(cf_s[:], o_r[:], r63_s[:, 0:1],
                                                None, op0=ALU.mult)
                        ci_s = eb.tile([128, C], i32, tag="ci")
                        nc.vector.tensor_copy(ci_s[:], cf_s[:])
                        cv = ci_s[:].rearrange("p (g j) -> p g j", j=4)
                        G32 = C // 4
                        t1_s = eb.tile([128, G32], i32, tag="t1")
                        t2_s = eb.tile([128, G32], i32, tag="t2")
                        b0_s = eb.tile([128, G32], i32, tag="b0")
                        b1_s = eb.tile([128, G32], i32, tag="b1")
                        b2_s = eb.tile([128, G32], i32, tag="b2")
                        # b0 = c0 | (c1 & 3) << 6
                        nc.vector.tensor_scalar(t1_s[:], cv[:, :, 1], 3, None,
                                                op0=ALU.bitwise_and)
                        nc.vector.tensor_scalar(t1_s[:], t1_s[:], 6, None,
                                                op0=ALU.logical_shift_left)
                        nc.vector.tensor_tensor(b0_s[:], cv[:, :, 0], t1_s[:],
                                                op=ALU.bitwise_or)
                        # b1 = (c1 >> 2) | (c2 & 15) << 4
                        nc.vector.tensor_scalar(t1_s[:], cv[:, :, 1], 2, None,
                                                op0=ALU.logical_shift_right)
                        nc.vector.tensor_scalar(t2_s[:], cv[:, :, 2], 15, None,
                                                op0=ALU.bitwise_and)
                        nc.vector.tensor_scalar(t2_s[:], t2_s[:], 4, None,
                                                op0=ALU.logical_shift_left)
                        nc.vector.tensor_tensor(b1_s[:], t1_s[:], t2_s[:],
                                                op=ALU.bitwise_or)
                        # b2 = (c2 >> 4) | c3 << 2
                        nc.vector.tensor_scalar(t1_s[:], cv[:, :, 2], 4, None,
                                                op0=ALU.logical_shift_right)
                        nc.vector.tensor_scalar(t2_s[:], cv[:, :, 3], 2, None,
                                                op0=ALU.logical_shift_left)
                        nc.vector.tensor_tensor(b2_s[:], t1_s[:], t2_s[:],
                                                op=ALU.bitwise_or)
                        pb_s = eb.tile([128, 96], mybir.dt.uint8, tag="pb")
                        pv = pb_s[:].rearrange("p (g j) -> p g j", j=3)
                        nc.vector.tensor_copy(pv[:, :, 0], b0_s[:])
                        nc.vector.tensor_copy(pv[:, :, 1], b1_s[:])
                        nc.vector.tensor_copy(pv[:, :, 2], b2_s[:])
                        msc_s = eb.tile([128, 1], f32, tag="msc")
                        nc.vector.tensor_scalar(msc_s[:], mx_s[:], 1.0 / 63.0,
                                                None, op0=ALU.mult)
                        nc.gpsimd.indirect_dma_start(
                            out=y_d[:],
                            out_offset=IndirectOffsetOnAxis(ap=or_s[:, :1], axis=0),
                            in_=pb_s[:], in_offset=None)
                        nc.gpsimd.indirect_dma_start(
                            out=ysc_d[:],
                            out_offset=IndirectOffsetOnAxis(ap=or_s[:, :1], axis=0),
                            in_=msc_s[:], in_offset=None)
                    else:
                        o2_s = eb.tile([128, C], mybir.dt.uint8, tag="o2")
                        # f32->u8 conversion rounds to nearest
                        nc.scalar.activation(o2_s[:], o_s[:], AF.Relu,
                                             scale=1.0 / cfg.Y_SCALE)
                        nc.gpsimd.indirect_dma_start(
                            out=y_d[:],
                            out_offset=IndirectOffsetOnAxis(ap=or_s[:, :1], axis=0),
                            in_=o2_s[:], in_offset=None)
    nc.finalize()
    return nc


def _build_inputs(inputs, cfg):
    x = np.ascontiguousarray(np.asarray(inputs["x"], np.float32))
    pos = np.ascontiguousarray(np.asarray(inputs["pos"], np.float32))
    W_lin = np.asarray(inputs["W_lin"], np.float32)
    W_src = np.asarray(inputs["W_src"], np.float32)
    W_dst = np.asarray(inputs["W_dst"], np.float32)
    Wp1 = np.asarray(inputs["Wp1"], np.float32)
    bp1 = np.asarray(inputs["bp1"], np.float32)
    Wp2 = np.asarray(inputs["Wp2"], np.float32)
    bp2 = np.asarray(inputs["bp2"], np.float32)
    Wa1 = np.asarray(inputs["Wa1"], np.float32)
    ba1 = np.asarray(inputs["ba1"], np.float32)
    Wa2 = np.asarray(inputs["Wa2"], np.float32)
    ba2 = np.asarray(inputs["ba2"], np.float32)

    Wda = (W_dst @ Wa1).astype(np.float32)   # [C, AH]
    Wsa = (W_src @ Wa1).astype(np.float32)
    wnode = np.concatenate([Wda, Wsa, W_lin], axis=1)  # [C, 2AH + C]
    bias = np.zeros((128, 5), np.float32)
    bias[: cfg.PH, 0] = bp1
    bias[: cfg.C, 1] = bp2
    bias[: cfg.AH, 2] = ba1
    bias[: cfg.C, 3] = ba2
    bias[:, 4] = -cfg.SHIFT

    # weight blob [128, KW]: wnode | Wa1 | Wp2^T | Wa2^T | bias | Wp1
    KW = cfg.M * cfg.KWSH
    wblob = np.zeros((128, KW), np.float32)
    wblob[:, 0:256] = wnode
    wblob[:, 256:320] = Wa1
    wblob[:, 320:384] = Wp2.T
    wblob[:, 384:448] = Wa2.T
    wblob[:, 448:453] = bias
    wblob[0:2, 456:520] = Wp1 / np.float32(65535.0)
    wblob16 = _to_bf16_bits(wblob)

    # x int8 quantization with per-node scale
    xsc = np.maximum(np.abs(x).max(axis=1, keepdims=True), 1e-12) / 127.0
    xq = np.clip(np.round(x / xsc), -127, 127).astype(np.int8)
    # pos u16 codes; dequant folded into Wp1
    pq = np.clip(np.round(pos * 65535.0), 0, 65535).astype(np.uint16)

    packs, nchunk = _pack(x, pos, inputs["edge_index"], cfg)
    in_maps = []
    for c, p in enumerate(packs):
        sl = slice(c * cfg.NSH, (c + 1) * cfg.NSH)
        m = dict(wblob=np.ascontiguousarray(
                     wblob16[:, c * cfg.KWSH:(c + 1) * cfg.KWSH]),
                 xqsh=np.ascontiguousarray(xq[sl].T),
                 xsc=np.ascontiguousarray(xsc[sl].astype(np.float32)),
                 posT=np.ascontiguousarray(pq[sl].T),
                 basec=np.full((128, 1), c * cfg.NLOC, np.int32),
                 **p)
        in_maps.append(m)
    return in_maps, nchunk


def assemble_y(results, cfg=CFG):
    """Concatenate per-core outputs and decode to f32 [N, C]."""
    if cfg.y_mode == "u6":
        y6 = np.concatenate(
            [results[c]["y"][: cfg.NLOC] for c in range(cfg.M)], axis=0)
        ysc = np.concatenate(
            [results[c]["ysc"][: cfg.NLOC] for c in range(cfg.M)], axis=0)
        b = y6.reshape(-1, 32, 3).astype(np.int32)
        c0 = b[..., 0] & 63
        c1 = ((b[..., 0] >> 6) | (b[..., 1] << 2)) & 63
        c2 = ((b[..., 1] >> 4) | (b[..., 2] << 4)) & 63
        c3 = (b[..., 2] >> 2) & 63
        codes = np.stack([c0, c1, c2, c3], axis=-1).reshape(-1, CFG.C)
        return codes.astype(np.float32) * ysc.astype(np.float32)
    y = np.concatenate(
        [results[c]["y"][: cfg.NLOC] for c in range(cfg.M)], axis=0)
    return y.astype(np.float32) * np.float32(cfg.Y_SCALE)


# ---------------------------------------------------------------- runner
# Mirror of bass2jax.run_bass_via_pjrt, with two wall-clock fixes for the
# per-call path:
#   * the jitted shard_map executable is built ONCE and cached (the stock
#     helper re-jits a fresh closure every call -> ~1.3s of retrace/XLA
#     re-lowering per call),
#   * the zero-initialized ExternalOutput buffers are device-resident and
#     reused (not re-uploaded per call; the custom call copies them into
#     the result buffer device-side).
# Every call still ships all in_map bytes host->device, executes, and
# fetches the outputs back to numpy.
_RUNNER = {}


def _make_runner(nc, n_cores):
    import jax
    from jax.sharding import Mesh, PartitionSpec, NamedSharding
    from jax.experimental.shard_map import shard_map
    from concourse.bass2jax import (
        _bass_exec_p, partition_id_tensor, install_neuronx_cc_hook)

    install_neuronx_cc_hook()
    assert not nc.dbg_callbacks
    partition_name = (
        nc.partition_id_tensor.name if nc.partition_id_tensor else None)
    in_names, out_names, out_avals, zero_outs = [], [], [], []
    for alloc in nc.m.functions[0].allocations:
        if not isinstance(alloc, mybir.MemoryLocationSet):
            continue
        name = alloc.memorylocations[0].name
        if alloc.kind == "ExternalInput":
            if name != partition_name and name != (
                    nc.dbg_addr.name if nc.dbg_addr is not None else None):
                in_names.append(name)
        elif alloc.kind == "ExternalOutput":
            shape = tuple(alloc.tensor_shape)
            dtype = mybir.dt.np(alloc.dtype)
            out_avals.append(jax.core.ShapedArray(shape, dtype))
            zero_outs.append(np.zeros(shape, dtype))
            out_names.append(name)
    n_params = len(in_names)
    in_names_all = list(in_names) + out_names
    if nc.dbg_addr is not None:
        in_names_all.append(nc.dbg_addr.name)
    if partition_name is not None:
        in_names_all.append(partition_name)

    def _body(*args):
        operands = list(args)
        if nc.dbg_addr is not None:
            operands.append(jax.numpy.zeros((1, 2), jax.numpy.uint32))
        if partition_name is not None:
            operands.append(partition_id_tensor())
        return tuple(_bass_exec_p.bind(
            *operands, out_avals=tuple(out_avals),
            in_names=tuple(in_names_all), out_names=tuple(out_names),
            lowering_input_output_aliases=(),
            sim_require_finite=True, sim_require_nnan=True, nc=nc))

    devices = jax.devices()[:n_cores]
    mesh = Mesh(np.asarray(devices), ("core",))
    nsh = NamedSharding(mesh, PartitionSpec("core"))
    n_outs = len(out_avals)
    sharded = jax.jit(
        shard_map(_body, mesh=mesh,
                  in_specs=(PartitionSpec("core"),) * (n_params + n_outs),
                  out_specs=(PartitionSpec("core"),) * n_outs,
                  check_rep=False),
        keep_unused=True)
    dev_zeros = [
        jax.device_put(
            np.zeros((n_cores * z.shape[0], *z.shape[1:]), z.dtype), nsh)
        for z in zero_outs]

    import threading

    def run(in_maps):
        concat_in = [
            np.concatenate([np.asarray(m[nm]) for m in in_maps], axis=0)
            for nm in in_names]
        out_arrs = sharded(*concat_in, *dev_zeros)
        # fetch all output shards in parallel threads
        shard_bufs = []
        tasks = []
        for i in range(len(out_names)):
            shards = sorted(out_arrs[i].addressable_shards,
                            key=lambda s: s.index[0].start or 0)
            bufs = [None] * len(shards)
            shard_bufs.append(bufs)
            for j, sh_ in enumerate(shards):
                tasks.append((bufs, j, sh_))

        def grab(t):
            bufs, j, sh_ = t
            bufs[j] = np.asarray(sh_.data)

        ths = [threading.Thread(target=grab, args=(t,)) for t in tasks]
        for t in ths:
            t.start()
        for t in ths:
            t.join()
        return [
            {name: shard_bufs[i][c] for i, name in enumerate(out_names)}
            for c in range(n_cores)]

    return run


def run_cached(nc, in_maps, n_cores):
    key = id(nc)
    if key not in _RUNNER:
        _RUNNER[key] = _make_runner(nc, n_cores)
    return _RUNNER[key](in_maps)


def kernel(**inputs):
    cfg = CFG
    in_maps, nchunk = _build_inputs(inputs, cfg)
    nc = _build(cfg, nchunk)
    results = run_cached(nc, in_maps, cfg.M)
    return assemble_y(results, cfg)


# revision 19
# speedup vs baseline: 6.1115x; 1.0155x over previous
"""Trainium2 Bass kernel for nn_ClusterEncoder (PointTransformerConv-style
GNN message passing), 8-core SPMD.

The metric regime is axon-tunnel transfer-bound (tens of MB/s host<->device
plus ~80ms fixed dispatch), so the kernel minimizes per-call host bytes and
per-call Python/XLA overhead:
  * x ships as int8 with a per-node f32 scale (xT shard [128, 6250] i8 +
    scale [6250,1] f32 per core); full tables are assembled on device with
    AllGather collectives and the scale is applied post-matmul (U/V/H rows
    scale linearly in x_n).
  * pos ships as u16 codes (pos*65535), sharded + AllGathered; the 1/65535
    dequant is folded into Wp1 host-side.
  * No per-edge dst metadata: edges are sorted by dst and grouped into
    <=128-node chunks; the per-chunk edge->node indicator matrix is built
    on device from cumulative degrees (u16 [128,2] per chunk) and an iota,
    replacing the dstid/dstloc uploads. posd is never shipped: the pos MLP
    layer-1 (pos @ Wp1) is computed per node in phase 1 and distributed to
    edges through the same indicator matmul (it is linear, so
    P1[dst]-P1[src] can replace Wp1^T posd).
  * y returns as u8 (scale 2.75/255), decoded on host.
  * run_cached keeps ONE jitted shard_map executable alive across calls and
    keeps the zero-filled output donation buffers device-resident, so a
    steady-state call is: concat numpy inputs -> h2d -> exec -> d2h.

Compute strategy (edges sharded by destination node):
  * Host: sort edges by dst, split nodes into 8 equal contiguous ranges.
    Greedy-pack destination nodes into chunks of <=128 nodes / <=CHUNK_E
    edges; ship per-chunk src ids (u16), local out rows (u16), and
    cumulative-degree pairs (u16).
  * Device, phase 0: AllGather x int8 shards, x scales, posT u16 shards.
  * Device, phase 1 (replicated): per 128-node tile
      uvhp = [x@Wda | x@Wsa | x@Wlin | pos@Wp1'] (two matmuls into one
      PSUM tile), scale the x part per node, write tables
      G = [U | P1]  [N+1, 128] and S = [V | H | P1]  [N, 256] to DRAM.
  * Device, phase 2 (per chunk of 16 x 128-edge tiles):
      - gather G rows once per chunk by outrow (dst nodes), gather S rows
        per edge by src id,
      - indicator indT[n,e] = (e>=cum[n]) - (e>=cum[n+1]) from iota,
      - dst-side distribute: Du = G_U^T @ indT, Dp = G_P1^T @ indT,
      - t_p1 = relu(Dp - P1s^T + bp1);  delta = relu(Wp2^T t_p1 + bp2),
      - t_a = relu(Wa1^T delta + (Du - V^T) + ba1),
      - logits = relu(Wa2^T t_a + ba2);  e = exp(logits - SHIFT)
        (constant shift cancels in e/sum(e); logits are relu-bounded),
      - seg-sum via matmul: acc[n,0:128] += ind^T @ (e*(H[src]+delta))^T,
        acc[n,128:256] += ind^T @ e^T,
      - out = relu(NUM / (s + eps)); indirect-scatter rows to y (u8).
  * Softmax segments are core-local by construction (edges sharded by dst).
"""
import sys
from dataclasses import dataclass
from math import ceil

if "/opt/trn_rl_repo" not in sys.path:
    sys.path.insert(0, "/opt/trn_rl_repo")

import ml_dtypes
import numpy as np

import concourse.bass as bass
import concourse.mybir as mybir
import concourse.tile as tile
from concourse import bacc
from concourse.bass import IndirectOffsetOnAxis
from concourse.bass_utils import run_bass_kernel_spmd
from concourse.masks import make_identity

f32 = mybir.dt.float32
f32r = mybir.dt.float32r
bf16 = mybir.dt.bfloat16
i32 = mybir.dt.int32
u16 = mybir.dt.uint16
i8 = mybir.dt.int8
AF = mybir.ActivationFunctionType
ALU = mybir.AluOpType
BF16 = ml_dtypes.bfloat16


@dataclass
class Cfg:
    N: int = 50000
    C: int = 128
    PH: int = 64
    AH: int = 64
    DIM: int = 2
    M: int = 8            # cores
    T: int = 16           # 128-edge tiles per chunk
    TB: int = 4           # tiles per matmul block (block = 512 edges)
    SHIFT: float = 8.0
    EPS: float = 1e-12
    mm_dt: object = f32r  # matmul compute dtype (f32r: 1 cyc/row at free>=256)
    y_mode: str = "u6"    # "u8" | "u6" output encoding
    Y_SCALE: float = 2.75 / 255.0  # u8 quant step (y in [0, ~2.44])
    KWSH: int = 65        # weight-blob columns per core shard (8*65 = 520)

    @property
    def NLOC(self):
        return self.N // self.M

    @property
    def NSH(self):
        return self.N // self.M  # x rows per core shard

    @property
    def CHUNK_E(self):
        return self.T * 128

    @property
    def OUT_ROWS(self):
        return self.NLOC + 1  # +1 trash row for padded scatter lanes


CFG = Cfg()


def _to_bf16_bits(a):
    return np.ascontiguousarray(a.astype(BF16).view(np.uint16))


# ---------------------------------------------------------------- host pack
def _pack(x, pos, edge_index, cfg):
    """Sort/shard/chunk edges; returns per-core input dicts (minus weights)."""
    src = np.asarray(edge_index[0], np.int64)
    dst = np.asarray(edge_index[1], np.int64)
    order = np.argsort(dst, kind="stable")
    s_s = src[order]
    d_s = dst[order]

    NLOC = cfg.NLOC
    bounds = np.searchsorted(d_s, np.arange(cfg.M + 1) * NLOC)

    cores = []
    for c in range(cfg.M):
        lo, hi = bounds[c], bounds[c + 1]
        dloc = d_s[lo:hi] - c * NLOC
        deg = np.bincount(dloc, minlength=NLOC)
        nodes = np.nonzero(deg)[0]
        chunks = []  # (node_list, e0, e1) ; e relative to lo
        cur, cur_e, estart = [], 0, 0
        for n in nodes:
            dn = int(deg[n])
            assert dn <= cfg.CHUNK_E, f"degree {dn} exceeds chunk capacity"
            if len(cur) == 128 or cur_e + dn > cfg.CHUNK_E:
                chunks.append((cur, estart, estart + cur_e))
                estart += cur_e
                cur, cur_e = [], 0
            cur.append(int(n))
            cur_e += dn
        if cur:
            chunks.append((cur, estart, estart + cur_e))
        cores.append((lo, chunks, deg))

    NCHUNK = max(max(len(ch) for _, ch, _ in cores), 1)

    in_maps = []
    for c in range(cfg.M):
        lo, chunks, deg = cores[c]
        srcid = np.zeros((NCHUNK, 128, cfg.T), np.uint16)
        cum129 = np.zeros((NCHUNK, 129, 1), np.uint16)
        outrow = np.full((NCHUNK, 128), cfg.NLOC, np.uint16)  # trash row
        for k, (nl, e0, e1) in enumerate(chunks):
            cnt = e1 - e0
            g0, g1 = lo + e0, lo + e1
            nla = np.asarray(nl, np.int64)
            j = np.arange(cnt)
            srcid[k, j & 127, j >> 7] = s_s[g0:g1].astype(np.uint16)
            cums = np.concatenate([[0], np.cumsum(deg[nla])])
            cum129[k, :, 0] = cnt
            cum129[k, : len(nl) + 1, 0] = cums
            outrow[k, : len(nl)] = nla.astype(np.uint16)
        in_maps.append(dict(srcid=srcid, cum129=cum129, outrow=outrow))
    return in_maps, NCHUNK


# ---------------------------------------------------------------- program
def _build(cfg, nchunk):
    nc = bacc.Bacc(None, target_bir_lowering=False, num_devices=cfg.M)
    N, C, PH, AH, DIM = cfg.N, cfg.C, cfg.PH, cfg.AH, cfg.DIM
    NSH = cfg.NSH
    mdt = cfg.mm_dt

    u8 = mybir.dt.uint8
    KW = cfg.M * cfg.KWSH  # 520 weight-blob columns
    xq_d = nc.declare_dram_parameter("xqsh", [C, NSH], i8, isOutput=False)
    xsc_d = nc.declare_dram_parameter("xsc", [NSH, 1], bf16, isOutput=False)
    pos_d = nc.declare_dram_parameter("posT", [DIM, NSH], u8, isOutput=False)
    wb_d = nc.declare_dram_parameter("wblob", [128, cfg.KWSH], bf16, isOutput=False)
    src_d = nc.declare_dram_parameter("srcid", [nchunk, 128, cfg.T], u16, isOutput=False)
    cum_d = nc.declare_dram_parameter("cum129", [nchunk, 129, 1], u16, isOutput=False)
    or_d = nc.declare_dram_parameter("outrow", [nchunk, 128], u16, isOutput=False)
    base_d = nc.declare_dram_parameter("basec", [128, 1], i32, isOutput=False)
    if cfg.y_mode == "u6":
        y_d = nc.declare_dram_parameter("y", [cfg.OUT_ROWS, 96], u8, isOutput=True)
        ysc_d = nc.declare_dram_parameter("ysc", [cfg.OUT_ROWS, 1], f32, isOutput=True)
    else:
        y_d = nc.declare_dram_parameter("y", [cfg.OUT_ROWS, C], u8, isOutput=True)

    # node tables: G = [U | P1] (dst side), S = [V | H | P1] (src side)
    G_d = nc.dram_tensor("G", [N + 1, AH + PH], f32)
    S_d = nc.dram_tensor("S", [N, AH + C + PH], f32)
    xg_in = nc.dram_tensor("xg_in", [C, NSH], i8)
    xq_all = nc.dram_tensor("xq_all", [cfg.M * C, NSH], i8)
    sc_in = nc.dram_tensor("sc_in", [NSH, 1], bf16)
    sc_all = nc.dram_tensor("sc_all", [cfg.M * NSH, 1], bf16)
    pg_in = nc.dram_tensor("pg_in", [DIM, NSH], u8)
    pos_all = nc.dram_tensor("pos_all", [cfg.M * DIM, NSH], u8)
    wb_in = nc.dram_tensor("wb_in", [128, cfg.KWSH], bf16)
    wb_all = nc.dram_tensor("wb_all", [cfg.M * 128, cfg.KWSH], bf16)

    NB = cfg.T // cfg.TB  # blocks per chunk
    BLK = cfg.TB * 128
    NCOL = 2 * AH + C  # 256: U | V | H psum columns
    NPS = NCOL + PH    # 320: + P1

    with tile.TileContext(nc) as tc:
        with tc.tile_pool(name="const", bufs=1) as cp:
            base_s = cp.tile([128, 1], i32)
            nc.sync.dma_start(out=base_s[:], in_=base_d[:, :])
            ident_s = cp.tile([128, 128], f32)
            make_identity(nc, ident_s[:])
            ident_r = cp.tile([128, 128], mdt)
            nc.vector.tensor_copy(ident_r[:], ident_s[:])
            iot_i = cp.tile([128, cfg.CHUNK_E], i32)
            nc.gpsimd.iota(iot_i[:], pattern=[[1, cfg.CHUNK_E]], base=0,
                           channel_multiplier=0)
            iot_f = cp.tile([128, cfg.CHUNK_E], f32)
            nc.vector.tensor_copy(iot_f[:], iot_i[:])

            # ------------- phase 0: AllGather weights / x / scales / pos ---
            groups = [list(range(cfg.M))]
            nc.gpsimd.dma_start(out=wb_in[:, :], in_=wb_d[:, :])
            nc.gpsimd.collective_compute(
                "AllGather", mybir.AluOpType.bypass, replica_groups=groups,
                ins=[wb_in[:, :]], outs=[wb_all[:, :]])
            nc.gpsimd.dma_start(out=xg_in[:, :], in_=xq_d[:, :])
            nc.gpsimd.collective_compute(
                "AllGather", mybir.AluOpType.bypass, replica_groups=groups,
                ins=[xg_in[:, :]], outs=[xq_all[:, :]])
            nc.gpsimd.dma_start(out=sc_in[:, :], in_=xsc_d[:, :])
            nc.gpsimd.collective_compute(
                "AllGather", mybir.AluOpType.bypass, replica_groups=groups,
                ins=[sc_in[:, :]], outs=[sc_all[:, :]])
            nc.gpsimd.dma_start(out=pg_in[:, :], in_=pos_d[:, :])
            nc.gpsimd.collective_compute(
                "AllGather", mybir.AluOpType.bypass, replica_groups=groups,
                ins=[pg_in[:, :]], outs=[pos_all[:, :]])

            # reassemble the weight blob [128, KW] from the gathered shards
            # blob cols: 0:256 wnode | 256:320 wa1 | 320:384 wp2^T
            #            | 384:448 wa2^T | 448:453 bias | 456:520 wp1 (rows 0:2)
            wb_s = cp.tile([128, KW], bf16)
            for c8 in range(cfg.M):
                nc.sync.dma_start(
                    out=wb_s[:, c8 * cfg.KWSH:(c8 + 1) * cfg.KWSH],
                    in_=wb_all[c8 * 128:(c8 + 1) * 128, :])
            wnode_m = cp.tile([C, NCOL], mdt)
            nc.vector.tensor_copy(wnode_m[:], wb_s[:, 0:256])
            wa1_m = cp.tile([C, AH], mdt)
            nc.vector.tensor_copy(wa1_m[:], wb_s[:, 256:320])
            bias_s = cp.tile([128, 5], f32)
            nc.vector.tensor_copy(bias_s[:], wb_s[:, 448:453])
            wp1_m = cp.tile([DIM, PH], mdt)
            nc.vector.tensor_copy(wp1_m[:], wb_s[0:DIM, 456:520])
            wp2t_f = cp.tile([C, PH], f32)
            nc.vector.tensor_copy(wp2t_f[:], wb_s[:, 320:384])
            wa2t_f = cp.tile([C, AH], f32)
            nc.vector.tensor_copy(wa2t_f[:], wb_s[:, 384:448])
            wp2_m = cp.tile([PH, C], mdt)
            wa2_m = cp.tile([AH, C], mdt)
            with tc.tile_pool(name="wtp", bufs=1, space="PSUM") as wtp:
                wt_p = wtp.tile([128, 128], f32, tag="wt")
                nc.tensor.transpose(wt_p[:PH, :], wp2t_f[:], ident_s[:])
                nc.scalar.activation(wp2_m[:], wt_p[:PH, :], AF.Copy)
                wt2_p = wtp.tile([128, 128], f32, tag="wt2")
                nc.tensor.transpose(wt2_p[:AH, :], wa2t_f[:], ident_s[:])
                nc.scalar.activation(wa2_m[:], wt2_p[:AH, :], AF.Copy)

            # ------------- phase 1: node tables G / S -------------
            with tc.tile_pool(name="p1", bufs=3) as p1, \
                 tc.tile_pool(name="p1ps", bufs=2, space="PSUM") as p1ps:
                # zero the trash row N of G (gathered by padded outrow slots)
                zrow = p1.tile([1, AH + PH], f32, tag="zrow")
                nc.vector.memset(zrow[:], 0.0)
                nc.sync.dma_start(out=G_d[N:N + 1, :], in_=zrow[:])
                ncol = ceil(NSH / 128)
                for c8 in range(cfg.M):
                    for t in range(ncol):
                        j0 = t * 128
                        rows = min(128, NSH - j0)
                        r0 = c8 * NSH + j0
                        xt = p1.tile([C, 128], i8, tag="xt")
                        nc.sync.dma_start(
                            out=xt[:, :rows],
                            in_=xq_all[c8 * C:(c8 + 1) * C, j0:j0 + rows])
                        xt_m = p1.tile([C, 128], mdt, tag="xtm")
                        nc.vector.tensor_copy(xt_m[:, :rows], xt[:, :rows])
                        pt = p1.tile([DIM, 128], u8, tag="pt")
                        nc.sync.dma_start(
                            out=pt[:, :rows],
                            in_=pos_all[c8 * DIM:(c8 + 1) * DIM, j0:j0 + rows])
                        pt_m = p1.tile([DIM, 128], mdt, tag="ptm")
                        nc.vector.tensor_copy(pt_m[:, :rows], pt[:, :rows])
                        sc_b = p1.tile([128, 1], bf16, tag="scb")
                        nc.sync.dma_start(
                            out=sc_b[:rows, :], in_=sc_all[r0:r0 + rows, :])
                        sc_t = p1.tile([128, 1], f32, tag="sc")
                        nc.vector.tensor_copy(sc_t[:rows, :], sc_b[:rows, :])
                        ups = p1ps.tile([128, NPS], f32, tag="ups")
                        nc.tensor.matmul(ups[:rows, 0:NCOL], lhsT=xt_m[:, :rows],
                                         rhs=wnode_m[:], start=True, stop=True)
                        nc.tensor.matmul(ups[:rows, NCOL:NPS], lhsT=pt_m[:, :rows],
                                         rhs=wp1_m[:], start=True, stop=True)
                        uvp_s = p1.tile([128, NPS], f32, tag="uvp")
                        nc.vector.tensor_scalar(uvp_s[:rows, 0:NCOL],
                                                ups[:rows, 0:NCOL],
                                                sc_t[:rows, 0:1], None,
                                                op0=ALU.mult)
                        nc.vector.tensor_copy(uvp_s[:rows, NCOL:NPS],
                                              ups[:rows, NCOL:NPS])
                        # G = [U | P1], S = [V | H | P1] (psum cols 64:320)
                        nc.sync.dma_start(out=G_d[r0:r0 + rows, 0:AH],
                                          in_=uvp_s[:rows, 0:AH])
                        nc.sync.dma_start(out=G_d[r0:r0 + rows, AH:AH + PH],
                                          in_=uvp_s[:rows, NCOL:NPS])
                        nc.sync.dma_start(out=S_d[r0:r0 + rows, :],
                                          in_=uvp_s[:rows, AH:NPS])

            # ------------- phase 2: edges -------------
            with tc.tile_pool(name="eb", bufs=3) as eb, \
                 tc.tile_pool(name="ebg", bufs=3) as ebg, \
                 tc.tile_pool(name="ps_acc", bufs=1, space="PSUM") as ps_acc, \
                 tc.tile_pool(name="ps_du", bufs=1, space="PSUM") as ps_du, \
                 tc.tile_pool(name="ps_dp", bufs=1, space="PSUM") as ps_dp, \
                 tc.tile_pool(name="ps_b", bufs=1, space="PSUM") as ps_b, \
                 tc.tile_pool(name="ps_c", bufs=1, space="PSUM") as ps_c, \
                 tc.tile_pool(name="ps_n", bufs=1, space="PSUM") as ps_n, \
                 tc.tile_pool(name="ps_t", bufs=2, space="PSUM") as ps_t:
                for k in range(nchunk):
                    src16 = eb.tile([128, cfg.T], u16, tag="src16")
                    nc.sync.dma_start(out=src16[:], in_=src_d[k, :, :])
                    src_s = eb.tile([128, cfg.T], i32, tag="src")
                    nc.vector.tensor_copy(src_s[:], src16[:])
                    or16 = eb.tile([128, 1], u16, tag="or16")
                    nc.sync.dma_start(out=or16[:], in_=or_d[k, :, None])
                    or_s = eb.tile([128, 1], i32, tag="or")
                    nc.vector.tensor_copy(or_s[:], or16[:])
                    org_s = eb.tile([128, 1], i32, tag="org")
                    nc.vector.tensor_tensor(org_s[:], or_s[:], base_s[:],
                                            op=ALU.add)
                    cum16 = eb.tile([128, 2], u16, tag="cum16")
                    nc.sync.dma_start(out=cum16[:, 0:1], in_=cum_d[k, 0:128, :])
                    nc.sync.dma_start(out=cum16[:, 1:2], in_=cum_d[k, 1:129, :])
                    cum_f = eb.tile([128, 2], f32, tag="cum")
                    nc.vector.tensor_copy(cum_f[:], cum16[:])
                    gl_s = eb.tile([128, AH + PH], f32, tag="gl")
                    nc.gpsimd.indirect_dma_start(
                        out=gl_s[:], out_offset=None, in_=G_d[:],
                        in_offset=IndirectOffsetOnAxis(ap=org_s[:, 0:1], axis=0))
                    gl_m = eb.tile([128, AH + PH], mdt, tag="glm")
                    nc.vector.tensor_copy(gl_m[:], gl_s[:])

                    acc_p = ps_acc.tile([128, 2 * C], f32, tag="acc")

                    for b in range(NB):
                        esl = slice(b * BLK, (b + 1) * BLK)
                        # per-edge src gathers for this block
                        svhs = []
                        for tt in range(cfg.TB):
                            ti = b * cfg.TB + tt
                            svh_t = ebg.tile([128, AH + C + PH], f32, tag=f"svh{tt}")
                            nc.gpsimd.indirect_dma_start(
                                out=svh_t[:], out_offset=None, in_=S_d[:],
                                in_offset=IndirectOffsetOnAxis(
                                    ap=src_s[:, ti:ti + 1], axis=0))
                            svhs.append(svh_t)

                        # indicator indT[n, e] from cumulative degrees
                        ge_a = eb.tile([128, BLK], mdt, tag="gea")
                        nc.vector.tensor_scalar(ge_a[:], iot_f[:, esl],
                                                cum_f[:, 0:1], None, op0=ALU.is_ge)
                        ge_b = eb.tile([128, BLK], mdt, tag="geb")
                        nc.vector.tensor_scalar(ge_b[:], iot_f[:, esl],
                                                cum_f[:, 1:2], None, op0=ALU.is_ge)
                        indT_s = eb.tile([128, BLK], mdt, tag="indT")
                        nc.vector.tensor_tensor(indT_s[:], ge_a[:], ge_b[:],
                                                op=ALU.subtract)

                        # dst-side distribute: Du = U^T ind, Dp = P1^T ind
                        du_p = ps_du.tile([AH, BLK], f32, tag="du")
                        nc.tensor.matmul(du_p[:], lhsT=gl_m[:, 0:AH],
                                         rhs=indT_s[:], start=True, stop=True)
                        dp_p = ps_dp.tile([PH, BLK], f32, tag="dp")
                        nc.tensor.matmul(dp_p[:], lhsT=gl_m[:, AH:AH + PH],
                                         rhs=indT_s[:], start=True, stop=True)

                        # src-side transposes; gd = Du - V^T, pd1 = Dp - P1s^T
                        gd_s = eb.tile([AH, BLK], f32, tag="gd")
                        pd_s = eb.tile([PH, BLK], f32, tag="pd")
                        for tt in range(cfg.TB):
                            csl = slice(tt * 128, (tt + 1) * 128)
                            vT_p = ps_t.tile([128, 128], f32, tag="tr")
                            nc.tensor.transpose(vT_p[:AH, :], svhs[tt][:, 0:AH],
                                                ident_s[:])
                            vT_s = eb.tile([AH, 128], f32, tag="vT")
                            nc.scalar.activation(vT_s[:], vT_p[:AH, :], AF.Copy)
                            nc.vector.tensor_tensor(gd_s[:, csl], du_p[:, csl],
                                                    vT_s[:], op=ALU.subtract)
                            pT_p = ps_t.tile([128, 128], f32, tag="tr")
                            nc.tensor.transpose(
                                pT_p[:PH, :], svhs[tt][:, AH + C:AH + C + PH],
                                ident_s[:])
                            pT_s = eb.tile([PH, 128], f32, tag="pT")
                            nc.scalar.activation(pT_s[:], pT_p[:PH, :], AF.Copy)
                            nc.vector.tensor_tensor(pd_s[:, csl], dp_p[:, csl],
                                                    pT_s[:], op=ALU.subtract)

                        # pos MLP layer 2 (layer 1 was folded through tables)
                        tp1_s = eb.tile([PH, BLK], mdt, tag="tp1")
                        nc.scalar.activation(tp1_s[:], pd_s[:], AF.Relu,
                                             bias=bias_s[0:PH, 0:1])
                        del_p = ps_b.tile([C, BLK], f32, tag="delp")
                        nc.tensor.matmul(del_p[:], lhsT=wp2_m[:],
                                         rhs=tp1_s[:], start=True, stop=True)
                        del_s = eb.tile([C, BLK], f32, tag="dels")
                        nc.scalar.activation(del_s[:], del_p[:], AF.Relu,
                                             bias=bias_s[:, 1:2])
                        del_m = eb.tile([C, BLK], mdt, tag="delm")
                        nc.scalar.activation(del_m[:], del_p[:], AF.Relu,
                                             bias=bias_s[:, 1:2])

                        # attn layer 1: t_a = relu(Wa1^T delta + gd + ba1)
                        z1_p = ps_n.tile([AH, BLK], f32, tag="z1")
                        nc.tensor.matmul(z1_p[:], lhsT=wa1_m[:],
                                         rhs=del_m[:], start=True, stop=True)
                        tsum_s = eb.tile([AH, BLK], f32, tag="tsum")
                        nc.vector.tensor_tensor(tsum_s[:], z1_p[:], gd_s[:],
                                                op=ALU.add)
                        ta_s = eb.tile([AH, BLK], mdt, tag="ta")
                        nc.scalar.activation(ta_s[:], tsum_s[:], AF.Relu,
                                             bias=bias_s[0:AH, 2:3])

                        # attn layer 2 + exp
                        al_p = ps_c.tile([C, BLK], f32, tag="al")
                        nc.tensor.matmul(al_p[:], lhsT=wa2_m[:],
                                         rhs=ta_s[:], start=True, stop=True)
                        ar_s = eb.tile([C, BLK], f32, tag="ar")
                        nc.scalar.activation(ar_s[:], al_p[:], AF.Relu,
                                             bias=bias_s[:, 3:4])
                        e_s = eb.tile([C, BLK], f32, tag="e")
                        nc.scalar.activation(e_s[:], ar_s[:], AF.Exp,
                                             bias=bias_s[:, 4:5])
                        ew2_s = eb.tile([C, BLK], f32, tag="ew2")
                        nc.vector.tensor_tensor(ew2_s[:], e_s[:], del_s[:],
                                                op=ALU.mult)

                        # per-tile: transpose, assemble [ew | e]^T, seg-matmul
                        for tt in range(cfg.TB):
                            ti = b * cfg.TB + tt
                            csl = slice(tt * 128, (tt + 1) * 128)
                            eT_p = ps_t.tile([128, 128], f32, tag="tr")
                            nc.tensor.transpose(eT_p[:], e_s[:, csl], ident_s[:])
                            ew2T_p = ps_t.tile([128, 128], f32, tag="tr")
                            nc.tensor.transpose(ew2T_p[:], ew2_s[:, csl], ident_s[:])
                            iT_p = ps_t.tile([128, 128], mdt, tag="tr")
                            nc.tensor.transpose(iT_p[:], indT_s[:, csl], ident_r[:])
                            ind_s = eb.tile([128, 128], mdt, tag="ind")
                            nc.scalar.activation(ind_s[:], iT_p[:], AF.Copy)
                            ewe_s = eb.tile([128, 2 * C], mdt, tag="ewe")
                            nc.vector.tensor_copy(ewe_s[:, C:], eT_p[:])
                            tmp_s = eb.tile([128, C], f32, tag="tmp")
                            nc.vector.tensor_tensor(tmp_s[:], eT_p[:],
                                                    svhs[tt][:, AH:AH + C],
                                                    op=ALU.mult)
                            nc.vector.tensor_tensor(ewe_s[:, 0:C], tmp_s[:],
                                                    ew2T_p[:], op=ALU.add)
                            nc.tensor.matmul(acc_p[:], lhsT=ind_s[:],
                                             rhs=ewe_s[:],
                                             start=(ti == 0), stop=(ti == cfg.T - 1))

                    # finalize chunk
                    sp_s = eb.tile([128, C], f32, tag="sp")
                    nc.vector.tensor_scalar_add(sp_s[:], acc_p[:, C:], cfg.EPS)
                    rp_s = eb.tile([128, C], f32, tag="rp")
                    nc.vector.reciprocal(rp_s[:], sp_s[:])
                    o_s = eb.tile([128, C], f32, tag="o")
                    nc.vector.tensor_tensor(o_s[:], acc_p[:, 0:C], rp_s[:],
                                            op=ALU.mult)
                    if cfg.y_mode == "u6":
                        # per-row 6-bit encode: code = round(relu(o)*63/rowmax),
                        # pack 4 codes into 3 bytes; ship rowmax/63 as f32.
                        o_r = eb.tile([128, C], f32, tag="orr")
                        nc.scalar.activation(o_r[:], o_s[:], AF.Relu)
                        mx_s = eb.tile([128, 1], f32, tag="mx")
                        nc.vector.reduce_max(mx_s[:], o_r[:],
                                             axis=mybir.AxisListType.X)
                        mp_s = eb.tile([128, 1], f32, tag="mp")
                        nc.vector.tensor_scalar_add(mp_s[:], mx_s[:], 1e-30)
                        rq_s = eb.tile([128, 1], f32, tag="rq")
                        nc.vector.reciprocal(rq_s[:], mp_s[:])
                        r63_s = eb.tile([128, 1], f32, tag="r63")
                        nc.vector.tensor_scalar(r63_s[:], rq_s[:], 63.0, None,
                                                op0=ALU.mult)
                        cf_s = eb.tile([128, C], f32, tag="cf")
                        nc.vector.tensor_scalar(cf_s[:], o_r[:], r63_s[:, 0:1],
                                                None, op0=ALU.mult)
                        ci_s = eb.tile([128, C], i32, tag="ci")
                        nc.vector.tensor_copy(ci_s[:], cf_s[:])
                        cv = ci_s[:].rearrange("p (g j) -> p g j", j=4)
                        G32 = C // 4
                        t1_s = eb.tile([128, G32], i32, tag="t1")
                        t2_s = eb.tile([128, G32], i32, tag="t2")
                        b0_s = eb.tile([128, G32], i32, tag="b0")
                        b1_s = eb.tile([128, G32], i32, tag="b1")
                        b2_s = eb.tile([128, G32], i32, tag="b2")
                        # b0 = c0 | (c1 & 3) << 6
                        nc.vector.tensor_scalar(t1_s[:], cv[:, :, 1], 3, None,
                                                op0=ALU.bitwise_and)
                        nc.vector.tensor_scalar(t1_s[:], t1_s[:], 6, None,
                                                op0=ALU.logical_shift_left)
                        nc.vector.tensor_tensor(b0_s[:], cv[:, :, 0], t1_s[:],
                                                op=ALU.bitwise_or)
                        # b1 = (c1 >> 2) | (c2 & 15) << 4
                        nc.vector.tensor_scalar(t1_s[:], cv[:, :, 1], 2, None,
                                                op0=ALU.logical_shift_right)
                        nc.vector.tensor_scalar(t2_s[:], cv[:, :, 2], 15, None,
                                                op0=ALU.bitwise_and)
                        nc.vector.tensor_scalar(t2_s[:], t2_s[:], 4, None,
                                                op0=ALU.logical_shift_left)
                        nc.vector.tensor_tensor(b1_s[:], t1_s[:], t2_s[:],
                                                op=ALU.bitwise_or)
                        # b2 = (c2 >> 4) | c3 << 2
                        nc.vector.tensor_scalar(t1_s[:], cv[:, :, 2], 4, None,
                                                op0=ALU.logical_shift_right)
                        nc.vector.tensor_scalar(t2_s[:], cv[:, :, 3], 2, None,
                                                op0=ALU.logical_shift_left)
                        nc.vector.tensor_tensor(b2_s[:], t1_s[:], t2_s[:],
                                                op=ALU.bitwise_or)
                        pb_s = eb.tile([128, 96], mybir.dt.uint8, tag="pb")
                        pv = pb_s[:].rearrange("p (g j) -> p g j", j=3)
                        nc.vector.tensor_copy(pv[:, :, 0], b0_s[:])
                        nc.vector.tensor_copy(pv[:, :, 1], b1_s[:])
                        nc.vector.tensor_copy(pv[:, :, 2], b2_s[:])
                        msc_s = eb.tile([128, 1], f32, tag="msc")
                        nc.vector.tensor_scalar(msc_s[:], mx_s[:], 1.0 / 63.0,
                                                None, op0=ALU.mult)
                        nc.gpsimd.indirect_dma_start(
                            out=y_d[:],
                            out_offset=IndirectOffsetOnAxis(ap=or_s[:, :1], axis=0),
                            in_=pb_s[:], in_offset=None)
                        nc.gpsimd.indirect_dma_start(
                            out=ysc_d[:],
                            out_offset=IndirectOffsetOnAxis(ap=or_s[:, :1], axis=0),
                            in_=msc_s[:], in_offset=None)
                    else:
                        o2_s = eb.tile([128, C], mybir.dt.uint8, tag="o2")
                        # f32->u8 conversion rounds to nearest
                        nc.scalar.activation(o2_s[:], o_s[:], AF.Relu,
                                             scale=1.0 / cfg.Y_SCALE)
                        nc.gpsimd.indirect_dma_start(
                            out=y_d[:],
                            out_offset=IndirectOffsetOnAxis(ap=or_s[:, :1], axis=0),
                            in_=o2_s[:], in_offset=None)
    nc.finalize()
    return nc


def _build_inputs(inputs, cfg):
    x = np.ascontiguousarray(np.asarray(inputs["x"], np.float32))
    pos = np.ascontiguousarray(np.asarray(inputs["pos"], np.float32))
    W_lin = np.asarray(inputs["W_lin"], np.float32)
    W_src = np.asarray(inputs["W_src"], np.float32)
    W_dst = np.asarray(inputs["W_dst"], np.float32)
    Wp1 = np.asarray(inputs["Wp1"], np.float32)
    bp1 = np.asarray(inputs["bp1"], np.float32)
    Wp2 = np.asarray(inputs["Wp2"], np.float32)
    bp2 = np.asarray(inputs["bp2"], np.float32)
    Wa1 = np.asarray(inputs["Wa1"], np.float32)
    ba1 = np.asarray(inputs["ba1"], np.float32)
    Wa2 = np.asarray(inputs["Wa2"], np.float32)
    ba2 = np.asarray(inputs["ba2"], np.float32)

    Wda = (W_dst @ Wa1).astype(np.float32)   # [C, AH]
    Wsa = (W_src @ Wa1).astype(np.float32)
    wnode = np.concatenate([Wda, Wsa, W_lin], axis=1)  # [C, 2AH + C]
    bias = np.zeros((128, 5), np.float32)
    bias[: cfg.PH, 0] = bp1
    bias[: cfg.C, 1] = bp2
    bias[: cfg.AH, 2] = ba1
    bias[: cfg.C, 3] = ba2
    bias[:, 4] = -cfg.SHIFT

    # weight blob [128, KW]: wnode | Wa1 | Wp2^T | Wa2^T | bias | Wp1
    KW = cfg.M * cfg.KWSH
    wblob = np.zeros((128, KW), np.float32)
    wblob[:, 0:256] = wnode
    wblob[:, 256:320] = Wa1
    wblob[:, 320:384] = Wp2.T
    wblob[:, 384:448] = Wa2.T
    wblob[:, 448:453] = bias
    wblob[0:2, 456:520] = Wp1 / np.float32(255.0)
    wblob16 = _to_bf16_bits(wblob)

    # x int8 quantization with per-node scale
    xsc = np.maximum(np.abs(x).max(axis=1, keepdims=True), 1e-12) / 127.0
    xq = np.clip(np.round(x / xsc), -127, 127).astype(np.int8)
    # pos u16 codes; dequant folded into Wp1
    pq = np.clip(np.round(pos * 255.0), 0, 255).astype(np.uint8)

    packs, nchunk = _pack(x, pos, inputs["edge_index"], cfg)
    in_maps = []
    for c, p in enumerate(packs):
        sl = slice(c * cfg.NSH, (c + 1) * cfg.NSH)
        m = dict(wblob=np.ascontiguousarray(
                     wblob16[:, c * cfg.KWSH:(c + 1) * cfg.KWSH]),
                 xqsh=np.ascontiguousarray(xq[sl].T),
                 xsc=_to_bf16_bits(xsc[sl].astype(np.float32)),
                 posT=np.ascontiguousarray(pq[sl].T),
                 basec=np.full((128, 1), c * cfg.NLOC, np.int32),
                 **p)
        in_maps.append(m)
    return in_maps, nchunk


def assemble_y(results, cfg=CFG):
    """Concatenate per-core outputs and decode to f32 [N, C]."""
    if cfg.y_mode == "u6":
        y6 = np.concatenate(
            [results[c]["y"][: cfg.NLOC] for c in range(cfg.M)], axis=0)
        ysc = np.concatenate(
            [results[c]["ysc"][: cfg.NLOC] for c in range(cfg.M)], axis=0)
        b = y6.reshape(-1, 32, 3).astype(np.int32)
        c0 = b[..., 0] & 63
        c1 = ((b[..., 0] >> 6) | (b[..., 1] << 2)) & 63
        c2 = ((b[..., 1] >> 4) | (b[..., 2] << 4)) & 63
        c3 = (b[..., 2] >> 2) & 63
        codes = np.stack([c0, c1, c2, c3], axis=-1).reshape(-1, CFG.C)
        return codes.astype(np.float32) * ysc.astype(np.float32)
    y = np.concatenate(
        [results[c]["y"][: cfg.NLOC] for c in range(cfg.M)], axis=0)
    return y.astype(np.float32) * np.float32(cfg.Y_SCALE)


# ---------------------------------------------------------------- runner
# Mirror of bass2jax.run_bass_via_pjrt, with two wall-clock fixes for the
# per-call path:
#   * the jitted shard_map executable is built ONCE and cached (the stock
#     helper re-jits a fresh closure every call -> ~1.3s of retrace/XLA
#     re-lowering per call),
#   * the zero-initialized ExternalOutput buffers are device-resident and
#     reused (not re-uploaded per call; the custom call copies them into
#     the result buffer device-side).
# Every call still ships all in_map bytes host->device, executes, and
# fetches the outputs back to numpy.
_RUNNER = {}


def _make_runner(nc, n_cores):
    import jax
    from jax.sharding import Mesh, PartitionSpec, NamedSharding
    from jax.experimental.shard_map import shard_map
    from concourse.bass2jax import (
        _bass_exec_p, partition_id_tensor, install_neuronx_cc_hook)

    install_neuronx_cc_hook()
    assert not nc.dbg_callbacks
    partition_name = (
        nc.partition_id_tensor.name if nc.partition_id_tensor else None)
    in_names, out_names, out_avals, zero_outs = [], [], [], []
    for alloc in nc.m.functions[0].allocations:
        if not isinstance(alloc, mybir.MemoryLocationSet):
            continue
        name = alloc.memorylocations[0].name
        if alloc.kind == "ExternalInput":
            if name != partition_name and name != (
                    nc.dbg_addr.name if nc.dbg_addr is not None else None):
                in_names.append(name)
        elif alloc.kind == "ExternalOutput":
            shape = tuple(alloc.tensor_shape)
            dtype = mybir.dt.np(alloc.dtype)
            out_avals.append(jax.core.ShapedArray(shape, dtype))
            zero_outs.append(np.zeros(shape, dtype))
            out_names.append(name)
    n_params = len(in_names)
    in_names_all = list(in_names) + out_names
    if nc.dbg_addr is not None:
        in_names_all.append(nc.dbg_addr.name)
    if partition_name is not None:
        in_names_all.append(partition_name)

    def _body(*args):
        operands = list(args)
        if nc.dbg_addr is not None:
            operands.append(jax.numpy.zeros((1, 2), jax.numpy.uint32))
        if partition_name is not None:
            operands.append(partition_id_tensor())
        return tuple(_bass_exec_p.bind(
            *operands, out_avals=tuple(out_avals),
            in_names=tuple(in_names_all), out_names=tuple(out_names),
            lowering_input_output_aliases=(),
            sim_require_finite=True, sim_require_nnan=True, nc=nc))

    devices = jax.devices()[:n_cores]
    mesh = Mesh(np.asarray(devices), ("core",))
    nsh = NamedSharding(mesh, PartitionSpec("core"))
    n_outs = len(out_avals)
    sharded = jax.jit(
        shard_map(_body, mesh=mesh,
                  in_specs=(PartitionSpec("core"),) * (n_params + n_outs),
                  out_specs=(PartitionSpec("core"),) * n_outs,
                  check_rep=False),
        keep_unused=True)
    dev_zeros = [
        jax.device_put(
            np.zeros((n_cores * z.shape[0], *z.shape[1:]), z.dtype), nsh)
        for z in zero_outs]

    import threading

    def run(in_maps):
        concat_in = [
            np.concatenate([np.asarray(m[nm]) for m in in_maps], axis=0)
            for nm in in_names]
        out_arrs = sharded(*concat_in, *dev_zeros)
        # fetch all output shards in parallel threads
        shard_bufs = []
        tasks = []
        for i in range(len(out_names)):
            shards = sorted(out_arrs[i].addressable_shards,
                            key=lambda s: s.index[0].start or 0)
            bufs = [None] * len(shards)
            shard_bufs.append(bufs)
            for j, sh_ in enumerate(shards):
                tasks.append((bufs, j, sh_))

        def grab(t):
            bufs, j, sh_ = t
            bufs[j] = np.asarray(sh_.data)

        ths = [threading.Thread(target=grab, args=(t,)) for t in tasks]
        for t in ths:
            t.start()
        for t in ths:
            t.join()
        return [
            {name: shard_bufs[i][c] for i, name in enumerate(out_names)}
            for c in range(n_cores)]

    return run


def run_cached(nc, in_maps, n_cores):
    key = id(nc)
    if key not in _RUNNER:
        _RUNNER[key] = _make_runner(nc, n_cores)
    return _RUNNER[key](in_maps)


def kernel(**inputs):
    cfg = CFG
    in_maps, nchunk = _build_inputs(inputs, cfg)
    nc = _build(cfg, nchunk)
    results = run_cached(nc, in_maps, cfg.M)
    return assemble_y(results, cfg)
